# revision 16
# baseline (speedup 1.0000x reference)
"""Trainium2 Bass kernel: 3-layer LSTM LM (embed -> 3xLSTM(H=256) -> FC 32000 -> log_softmax).

Strategy: data-parallel over batch across 8 cores (2 sequences per core).

v2 wave design:
- LSTM cell reformulated with tanh only: sigmoid(x) = (1+tanh(x/2))/2 with the
  1/2 folded into pre-scaled weights, and doubled states c' = 2c, h' = 2h
  (weights consuming h are pre-halved).  This keeps the whole recurrent chain
  in the `exp_and_others` ACT table set, so log-softmax exp-accumulation runs
  inline during the wavefront with no table switches.
- The per-wave nonlinear chain is split into two independent streams
  ({L0,L1} merged via strided APs, {L2}) so each stream's chain overlaps the
  other stream's matmuls of the next wave.
- Cell update uses fused scalar_tensor_tensor ops:
    T = tanh(gates)            (ACT, all 16 cols/layer; weights pre-scaled)
    [p2,p1] = (T[i,f] + 1) * [T_g, c']      (one strided STT)
    c'_new  = 0.5*p1 + p2                    (STT)
    tc      = tanh(0.5 * c'_new)             (ACT, free input scale)
    h'      = (T_o + 1) * tc                 (STT)
- FC logits: matmul -> PSUM; exp-accumulated directly from PSUM (scalar) and
  staged to DRAM fp16 via gpsimd cast-DMA (no stage copies).  lse for token
  tiles 0/1 computed inline (Ln costs a table-switch pair each); their
  subtract+rewrite also runs inline.  Tiles 2/3 finish in the tail.
"""

import sys

sys.path.insert(0, "/opt/trn_rl_repo")

import numpy as np

import concourse.bass as bass
import concourse.mybir as mybir
import concourse.tile as tile
from concourse import bacc
from concourse.bass_utils import run_bass_kernel_spmd
from concourse.masks import make_identity
from concourse.tile import add_dep_helper

# Problem dims
V = 32000
E = 200
H = 256
B = 16
T = 256
N_CORES = 8
B_LOC = B // N_CORES  # 2 sequences per core
Bb = B_LOC
G4 = 4 * H  # 1024 gate width
TB = T * Bb  # 512 token-cols per h chunk

LAG = 16     # inter-layer lag in steps
SUB = 8      # xg-precompute granularity (steps per psum sub-chunk)
NSUB = T // SUB
W_TOT = T + 2 * LAG  # 288 waves

VC = 500           # FC vocab chunk (one PSUM bank)
NVC = V // VC      # 64
RBW = 2000         # readback/subtract chunk
NRB = V // RBW     # 16

FP16 = mybir.dt.float16
FP32 = mybir.dt.float32
AF = mybir.ActivationFunctionType
ALU = mybir.AluOpType
LAYER_DIMS = [E, H, H]


def mkap(tile_ap, off, dims):
    """Custom strided AP on a tile: off in elements, dims=[[step,count],...]."""
    return bass.AP(tile_ap.tensor, off,
                   [list(tile_ap.ap[0])] + [list(d) for d in dims])


def ksizes(dim):
    out = []
    while dim > 0:
        out.append(min(dim, 128))
        dim -= 128
    return out


def build_nc(has_lstm_bias=False, has_fc_bias=False):
    ntok = T * Bb  # 512 tokens per core
    n_mt = ntok // 128  # 4 fc token tiles

    nc = bacc.Bacc("TRN2", target_bir_lowering=False, debug=False,
                   num_devices=N_CORES)

    xids_d = nc.dram_tensor("xids", [ntok, 1], mybir.dt.int32, kind="ExternalInput")
    emb_d = nc.dram_tensor("emb", [V, E], FP32, kind="ExternalInput")
    wiT_d = [nc.dram_tensor(f"wiT{l}", [LAYER_DIMS[l], G4], FP16, kind="ExternalInput")
             for l in range(3)]
    whT_d = [nc.dram_tensor(f"whT{l}", [H, G4], FP16, kind="ExternalInput")
             for l in range(3)]
    bvec_d = [nc.dram_tensor(f"bvec{l}", [1, G4], FP16, kind="ExternalInput")
              for l in range(3)]
    fcWT_d = nc.dram_tensor("fcWT", [H, V], FP16, kind="ExternalInput")
    fcb_d = nc.dram_tensor("fcb", [1, V], FP16, kind="ExternalInput")
    out_d = nc.dram_tensor("out", [ntok, V], FP16, kind="ExternalOutput")

    with tile.TileContext(nc, num_cores=N_CORES) as tc:
        with (
            tc.tile_pool(name="state", bufs=1) as spool,
            tc.tile_pool(name="work", bufs=4) as work,
        ):
            # ---- persistent state ----
            hta = spool.tile([128, 6 * TB], FP16, tag="hta", name="hta")

            def ht_off(l, kc):
                return l * 2 * TB + kc * TB
            # cg: per layer 20 cols: [T16 (i,f,o,g tanh outputs) | c' (4)]
            cg = spool.tile([128, 60], FP32, tag="cg", name="cg")
            nc.vector.memset(cg[:], 0.0)
            zacc = spool.tile([128, 4 * NVC], FP32, tag="zacc", name="zacc")
            neglse = spool.tile([128, 4], FP32, tag="neglse", name="neglse")
            etrash = spool.tile([128, VC], FP16, tag="etrash", name="etrash")

            fcwpool = tc.alloc_tile_pool(name="fcw", bufs=1)
            fcw_sb = [fcwpool.tile([128, V], FP16, tag=f"fcw{kc}", name=f"fcw{kc}")
                      for kc in range(2)]
            fcb_sb = None
            if has_fc_bias:
                fcb_sb = fcwpool.tile([1, V], FP16, tag="fcb", name="fcb")

            stpool = tc.alloc_tile_pool(name="stage", bufs=4)
            rpool = tc.alloc_tile_pool(name="rb", bufs=4)
            wpool = tc.alloc_tile_pool(name="weights", bufs=1)
            # ---- Phase 0: LSTM weights to SBUF ----
            wiT_sb = []
            whT_sb = []
            bvec_sb = []
            for l in range(3):
                ks = ksizes(LAYER_DIMS[l])
                wi = wpool.tile([128, len(ks) * G4], FP16, tag=f"wiT{l}",
                                name=f"wiT{l}")
                for kc, ksz in enumerate(ks):
                    nc.sync.dma_start(
                        wi[0:ksz, kc * G4:(kc + 1) * G4],
                        wiT_d[l][kc * 128:kc * 128 + ksz, :],
                    )
                wiT_sb.append(wi)
                wh = wpool.tile([128, 2 * G4], FP16, tag=f"whT{l}", name=f"whT{l}")
                for kc in range(2):
                    nc.sync.dma_start(
                        wh[:, kc * G4:(kc + 1) * G4],
                        whT_d[l][kc * 128:(kc + 1) * 128, :],
                    )
                whT_sb.append(wh)
                if has_lstm_bias:
                    bv = wpool.tile([1, G4], FP16, tag=f"bvec{l}", name=f"bvec{l}")
                    nc.sync.dma_start(bv[:], bvec_d[l][:])
                    bvec_sb.append(bv)
                else:
                    bvec_sb.append(None)

            ones_sb = wpool.tile([1, 16], FP16, tag="ones", name="ones")
            nc.vector.memset(ones_sb[:], 1.0)
            onesf = spool.tile([1, 128], FP32, tag="onesf", name="onesf")
            nc.vector.memset(onesf[:], 1.0)
            ident = wpool.tile([128, 128], FP32, tag="ident", name="ident")
            make_identity(nc, ident[:])
            zrhs = wpool.tile([128, Bb], FP16, tag="zrhs", name="zrhs")
            nc.vector.memset(zrhs[:], 0.0)
            xT = wpool.tile([128, 2 * TB], FP16, tag="xT", name="xT")

            # ---- Phase 1: embedding gather + transpose into xT ----
            eks = ksizes(E)
            with tc.tile_pool(name="embps", bufs=2, space="PSUM") as eps:
                for gt in range(ntok // 128):
                    idt = work.tile([128, 1], mybir.dt.int32, tag="ids", name="ids")
                    nc.sync.dma_start(idt[:], xids_d[gt * 128:(gt + 1) * 128, :])
                    gat = work.tile([128, E], FP32, tag="gather", name="gather", bufs=2)
                    nc.gpsimd.indirect_dma_start(
                        out=gat[:],
                        out_offset=None,
                        in_=emb_d[:, :],
                        in_offset=bass.IndirectOffsetOnAxis(ap=idt[:, :1], axis=0),
                    )
                    for kc, ksz in enumerate(eks):
                        tp = eps.tile([128, 128], FP32, tag="tpsum", name="tpsum")
                        nc.tensor.transpose(
                            tp[0:ksz, 0:128],
                            gat[:, kc * 128:kc * 128 + ksz],
                            ident[:],
                        )
                        nc.vector.tensor_copy(
                            xT[0:ksz, kc * TB + gt * 128:kc * TB + (gt + 1) * 128],
                            tp[0:ksz, 0:128],
                        )

            # ---- Phase 2: wavefront with 2 chain streams + inline FC ----
            fcpp = tc.alloc_tile_pool(name="fcps", bufs=2, space="PSUM")
            gpp = tc.alloc_tile_pool(name="gps", bufs=1, space="PSUM")
            gp = gpp.tile([128, 3072], FP32, tag="gp", name="gp")
            openers = {}

            def emit_xg_piece(l, c, j):
                """Emit 2 of the 16 xg matmuls for (layer l, sub-chunk c)."""
                ks = ksizes(LAYER_DIMS[l])
                sb = (c % 2) * 1536 + l * 512
                for t in (2 * j, 2 * j + 1):
                    m, kc = t % 8, t // 8
                    ksz = ks[kc]
                    if l == 0:
                        rhs = xT[0:ksz, kc * TB + c * 16:kc * TB + (c + 1) * 16]
                    else:
                        rhs = hta[0:ksz,
                                  ht_off(l - 1, kc) + c * 16:ht_off(l - 1, kc) + (c + 1) * 16]
                    out = mkap(gp[:], sb + m * Bb, [[16, SUB], [1, Bb]])
                    is_open = (m == 0 and kc == 0)
                    mm = nc.tensor.matmul(
                        out,
                        lhsT=wiT_sb[l][0:ksz,
                                       kc * G4 + m * 128:kc * G4 + (m + 1) * 128],
                        rhs=rhs,
                        start=is_open,
                        stop=False,
                        skip_group_check=True,
                    )
                    if is_open:
                        openers[(l, c)] = mm.ins
                    else:
                        add_dep_helper(mm.ins, openers[(l, c)], sync=False,
                                       reason="slot opener order")
                if has_lstm_bias and j == 7:
                    for m in range(8):
                        mm = nc.tensor.matmul(
                            mkap(gp[:], sb + m * Bb, [[16, SUB], [1, Bb]]),
                            lhsT=bvec_sb[l][:, m * 128:(m + 1) * 128],
                            rhs=ones_sb[:, 0:16],
                            start=False,
                            stop=False,
                            skip_group_check=True,
                        )
                        add_dep_helper(mm.ins, openers[(l, c)], sync=False,
                                       reason="slot opener order")

            stage_state = {}

            def fc_mm(mt, v):
                """FC matmul for token-tile mt, vocab cols [v*VC, +VC) -> PSUM."""
                vs = v * VC
                ps = fcpp.tile([128, VC], FP32, tag="fcpsum", name="fcpsum")
                stage_state.setdefault("ps", {})[(mt, v)] = ps
                for kc in range(2):
                    nc.tensor.matmul(
                        ps[:],
                        lhsT=hta[:, ht_off(2, kc) + mt * 128:ht_off(2, kc) + (mt + 1) * 128],
                        rhs=fcw_sb[kc][:, vs:vs + VC],
                        start=(kc == 0),
                        stop=(kc == 1 and not has_fc_bias),
                        skip_group_check=True,
                    )
                if has_fc_bias:
                    nc.tensor.matmul(
                        ps[:],
                        lhsT=onesf[:, 0:128],
                        rhs=fcb_sb[:, vs:vs + VC],
                        start=False,
                        stop=True,
                        skip_group_check=True,
                    )
                return ps

            def fc_exp(mt, v, ps):
                """exp-accum directly from PSUM (fills the scalar wait gap)."""
                nc.scalar.activation(
                    etrash[:], ps[:], AF.Exp,
                    accum_out=zacc[:, mt * NVC + v:mt * NVC + v + 1],
                )

            def fc_stage(mt, v, ps):
                """fp32 PSUM -> fp16 stage (V); every 4th chunk DMA stage out."""
                if v % 4 == 0:
                    stage_state["tile"] = stpool.tile([128, 4 * VC], FP16,
                                                      tag="fcstage", name="fcstage")
                st = stage_state["tile"]
                nc.vector.tensor_copy(st[:, (v % 4) * VC:(v % 4 + 1) * VC], ps[:])
                if v % 4 == 3:
                    k = v // 4
                    nc.sync.dma_start(
                        out_d[mt * 128:(mt + 1) * 128, k * 4 * VC:(k + 1) * 4 * VC],
                        st[:],
                    )

            def fc_chunk(mt, v):
                ps = fc_mm(mt, v)
                fc_exp(mt, v, ps)
                fc_stage(mt, v, ps)

            def emit_lse(mt):
                """zacc[mt] -> neglse[:, mt].  Ln costs a table-switch pair."""
                zs = work.tile([128, 1], FP32, tag="zsum", name="zsum")
                nc.vector.tensor_reduce(
                    zs[:], zacc[:, mt * NVC:(mt + 1) * NVC],
                    op=ALU.add, axis=mybir.AxisListType.X,
                )
                lse = work.tile([128, 1], FP32, tag="lse", name="lse")
                nc.scalar.activation(lse[:], zs[:], AF.Ln)
                nc.vector.tensor_scalar_mul(neglse[:, mt:mt + 1], lse[:], -1.0)

            def emit_sub(mt, k):
                """Read back out_d chunk, add -lse, rewrite."""
                rb = rpool.tile([128, RBW], FP16, tag="rb", name="rb")
                nc.sync.dma_start(
                    rb[:], out_d[mt * 128:(mt + 1) * 128, k * RBW:(k + 1) * RBW])
                nc.vector.tensor_scalar_add(rb[:], rb[:], neglse[:, mt:mt + 1])
                nc.sync.dma_start(
                    out_d[mt * 128:(mt + 1) * 128, k * RBW:(k + 1) * RBW], rb[:])

            def emit_wh(group, w, gbase):
                for l in group:
                    tl = w - LAG * l
                    sb = gbase + l * 512
                    for kc in range(2):
                        if tl == 0:
                            rhs = zrhs[:, 0:Bb]
                        else:
                            rhs = hta[:, ht_off(l, kc) + (tl - 1) * Bb:
                                      ht_off(l, kc) + tl * Bb]
                        for m in range(8):
                            nc.tensor.matmul(
                                gp[:, sb + m * Bb:sb + (m + 1) * Bb],
                                lhsT=whT_sb[l][:, kc * G4 + m * 128:
                                               kc * G4 + (m + 1) * 128],
                                rhs=rhs,
                                start=False,
                                stop=(kc == 1),
                                skip_group_check=True,
                            )

            def chain_part1(group, w, gbase, tag):
                """tanh of gates, then fused cell update -> new c'."""
                l0, nl = group[0], len(group)
                nc.scalar.activation(
                    mkap(cg[:], 20 * l0, [[20, nl], [1, 16]]),
                    mkap(gp[:], gbase + l0 * 512, [[512, nl], [1, 16]]),
                    AF.Tanh,
                )
                prod = work.tile([128, 24], FP32, tag="prod" + tag,
                                 name="prod" + tag, bufs=4)
                # [p2,p1] = (T[i,f] + 1) * [T_g, c']
                nc.vector.scalar_tensor_tensor(
                    mkap(prod[:], 8 * l0, [[8, nl], [1, 8]]),
                    mkap(cg[:], 20 * l0, [[20, nl], [1, 8]]),
                    1.0,
                    mkap(cg[:], 20 * l0 + 12, [[20, nl], [1, 8]]),
                    ALU.add, ALU.mult,
                )
                # c'_new = 0.5*p1 + p2
                nc.vector.scalar_tensor_tensor(
                    mkap(cg[:], 20 * l0 + 16, [[20, nl], [1, 4]]),
                    mkap(prod[:], 8 * l0 + 4, [[8, nl], [1, 4]]),
                    0.5,
                    mkap(prod[:], 8 * l0, [[8, nl], [1, 4]]),
                    ALU.mult, ALU.add,
                )
                return prod

            def chain_part2(group, w, tag):
                """tanh(c) and h' = (T_o + 1) * tanh(c)."""
                l0, nl = group[0], len(group)
                tct = work.tile([128, 12], FP32, tag="tct" + tag,
                                name="tct" + tag, bufs=4)
                nc.scalar.activation(
                    mkap(tct[:], 4 * l0, [[4, nl], [1, 4]]),
                    mkap(cg[:], 20 * l0 + 16, [[20, nl], [1, 4]]),
                    AF.Tanh, scale=0.5,
                )
                for l in group:
                    nc.vector.scalar_tensor_tensor(
                        mkap(hta[:], 992 * l + w * Bb, [[TB, 2], [1, Bb]]),
                        cg[:, 20 * l + 8:20 * l + 12],
                        1.0,
                        tct[:, 4 * l:4 * l + 4],
                        ALU.add, ALU.mult,
                    )

            # xg calendar
            xg_cal = {}
            prologue_xg = []
            for l in range(3):
                for c in range(NSUB):
                    for j in range(8):
                        if l == 0:
                            w = SUB * (c - 1) + j
                        else:
                            w = LAG * l + SUB * c - 9 + j
                        if w < 0:
                            prologue_xg.append((l, c, j))
                        else:
                            xg_cal.setdefault(w, []).append((l, c, j))

            # FC calendar: token-tile mt ready at wave 64*mt+95
            fc_cal = {}
            for mt in range(3):
                for v in range(NVC):
                    fc_cal.setdefault(64 * mt + 96 + v, []).append((mt, v))
            # lse for mt0/mt1 inline, a few waves after the last chunk's exp
            lse_cal = {64 * mt + 96 + NVC + 4: mt for mt in range(2)}
            # subtract calendar for mt0/mt1
            sub_cal = {}
            for mt in range(2):
                for k in range(NRB):
                    sub_cal.setdefault(64 * mt + 96 + NVC + 7 + 2 * k, []).append((mt, k))

            for (l, c, j) in prologue_xg:
                emit_xg_piece(l, c, j)

            # fcW load: after the embedding/weight DMAs (first needed ~wave 96)
            for kc in range(2):
                for q in range(8):
                    nc.sync.dma_start(
                        fcw_sb[kc][:, q * 4000:(q + 1) * 4000],
                        fcWT_d[kc * 128:(kc + 1) * 128, q * 4000:(q + 1) * 4000])
            if has_fc_bias:
                nc.sync.dma_start(fcb_sb[:], fcb_d[:])

            for w in range(W_TOT):
                active = [l for l in range(3) if 0 <= w - LAG * l < T]
                g01 = [l for l in active if l < 2]
                g2 = [l for l in active if l == 2]
                P = (w // SUB) % 2
                s8 = w % SUB
                gbase = P * 1536 + s8 * 16

                emit_wh(g01, w, gbase)
                emit_wh(g2, w, gbase)
                if g01:
                    chain_part1(g01, w, gbase, "01")
                if g2:
                    chain_part1(g2, w, gbase, "2")
                # FC matmul + exp: the exp lands between the groups' chain
                # ACTs in the scalar FIFO, filling its dependency-wait gap.
                fcs = fc_cal.get(w, ())
                for (mt, v) in fcs:
                    ps = fc_mm(mt, v)
                    fc_exp(mt, v, ps)
                if g01:
                    chain_part2(g01, w, "01")
                if g2:
                    chain_part2(g2, w, "2")
                # stage copies after the chain's V ops to avoid head-blocking
                for (mt, v) in fcs:
                    fc_stage(mt, v, stage_state["ps"][(mt, v)])
                for (l, c, j) in xg_cal.get(w, ()):
                    emit_xg_piece(l, c, j)
                if w in lse_cal:
                    emit_lse(lse_cal[w])
                for (mt, k) in sub_cal.get(w, ()):
                    emit_sub(mt, k)

            wpool.release()
            gpp.release()

            # ---- Tail: mt2 subtract (overlaps mt3 FC), mt3 FC, lse3, subs
            emit_lse(2)
            for k in range(NRB):
                emit_sub(2, k)
            for v in range(NVC):
                fc_chunk(3, v)
            emit_lse(3)
            for k in range(NRB):
                emit_sub(3, k)

            rpool.release()
            stpool.release()
            fcpp.release()
            fcwpool.release()

    nc.compile()
    return nc


_nc_cache = {}


def _get_nc(has_lstm_bias, has_fc_bias):
    key = (has_lstm_bias, has_fc_bias)
    if key not in _nc_cache:
        _nc_cache[key] = build_nc(has_lstm_bias, has_fc_bias)
    return _nc_cache[key]


def prep_inputs(x, emb, Wi, Wh, bb, fcW, fcb):
    """Host-side shard + repack. Returns in_maps for the 8 cores.

    Gate rows reordered to [i,f,o,g].  Row scale 0.5 on i,f,o (sigmoid via
    tanh(x/2)); h-consuming weights additionally halved (h' = 2h); fcW halved.
    """
    perm = np.concatenate([np.arange(0, 512), np.arange(768, 1024),
                           np.arange(512, 768)])  # i,f | o | g
    rowscale = np.ones((G4, 1), np.float32)
    rowscale[0:768] = 0.5  # i,f,o rows: tanh(pre/2)
    shared = {
        "emb": np.ascontiguousarray(emb.astype(np.float32)),
        "fcWT": np.ascontiguousarray((fcW.T * 0.5).astype(np.float16)),
        "fcb": np.ascontiguousarray(fcb[None, :].astype(np.float16)),
    }
    for l in range(3):
        inscale = 1.0 if l == 0 else 0.5  # layers 1,2 consume h' = 2h
        shared[f"wiT{l}"] = np.ascontiguousarray(
            (Wi[l][perm] * rowscale * inscale).T.astype(np.float16))
        shared[f"whT{l}"] = np.ascontiguousarray(
            (Wh[l][perm] * rowscale * 0.5).T.astype(np.float16))
        shared[f"bvec{l}"] = np.ascontiguousarray(
            (bb[l][perm] * rowscale[:, 0])[None, :].astype(np.float16))
    in_maps = []
    for c in range(N_CORES):
        x_loc = x[c * B_LOC:(c + 1) * B_LOC, :]
        xids = np.ascontiguousarray(
            x_loc.T.reshape(-1, 1).astype(np.int32))  # [(t b), 1]
        m = dict(shared)
        m["xids"] = xids
        in_maps.append(m)
    return in_maps


def kernel(x, emb, Wi0, Wh0, b0, Wi1, Wh1, b1, Wi2, Wh2, b2, fcW, fcb,
           trace=False):
    x = np.asarray(x)
    bbs = [np.asarray(b0), np.asarray(b1), np.asarray(b2)]
    has_lstm_bias = bool(any(np.any(b) for b in bbs))
    has_fc_bias = bool(np.any(np.asarray(fcb)))
    nc = _get_nc(has_lstm_bias, has_fc_bias)
    in_maps = prep_inputs(
        np.asarray(x), np.asarray(emb),
        [np.asarray(Wi0), np.asarray(Wi1), np.asarray(Wi2)],
        [np.asarray(Wh0), np.asarray(Wh1), np.asarray(Wh2)],
        bbs, np.asarray(fcW), np.asarray(fcb))
    res = run_bass_kernel_spmd(nc, in_maps, core_ids=list(range(N_CORES)),
                               trace=trace)
    out = np.empty((B, T, V), np.float32)
    for c in range(N_CORES):
        oc = res.results[c]["out"].astype(np.float32).reshape(T, B_LOC, V)
        out[c * B_LOC:(c + 1) * B_LOC] = oc.transpose(1, 0, 2)
    kernel.last_results = res
    return out


# revision 19
# speedup vs baseline: 1.0018x; 1.0018x over previous
"""Trainium2 Bass kernel: 3-layer LSTM LM (embed -> 3xLSTM(H=256) -> FC 32000 -> log_softmax).

Strategy: data-parallel over batch across 8 cores (2 sequences per core).

v2 wave design:
- LSTM cell reformulated with tanh only: sigmoid(x) = (1+tanh(x/2))/2 with the
  1/2 folded into pre-scaled weights, and doubled states c' = 2c, h' = 2h
  (weights consuming h are pre-halved).  This keeps the whole recurrent chain
  in the `exp_and_others` ACT table set, so log-softmax exp-accumulation runs
  inline during the wavefront with no table switches.
- The per-wave nonlinear chain is split into two independent streams
  ({L0,L1} merged via strided APs, {L2}) so each stream's chain overlaps the
  other stream's matmuls of the next wave.
- Cell update uses fused scalar_tensor_tensor ops:
    T = tanh(gates)            (ACT, all 16 cols/layer; weights pre-scaled)
    [p2,p1] = (T[i,f] + 1) * [T_g, c']      (one strided STT)
    c'_new  = 0.5*p1 + p2                    (STT)
    tc      = tanh(0.5 * c'_new)             (ACT, free input scale)
    h'      = (T_o + 1) * tc                 (STT)
- FC logits: matmul -> PSUM; exp-accumulated directly from PSUM (scalar) and
  staged to DRAM fp16 via gpsimd cast-DMA (no stage copies).  lse for token
  tiles 0/1 computed inline (Ln costs a table-switch pair each); their
  subtract+rewrite also runs inline.  Tiles 2/3 finish in the tail.
"""

import sys

sys.path.insert(0, "/opt/trn_rl_repo")

import numpy as np

import concourse.bass as bass
import concourse.mybir as mybir
import concourse.tile as tile
from concourse import bacc
from concourse.bass_utils import run_bass_kernel_spmd
from concourse.masks import make_identity
from concourse.tile import add_dep_helper

# Problem dims
V = 32000
E = 200
H = 256
B = 16
T = 256
N_CORES = 8
B_LOC = B // N_CORES  # 2 sequences per core
Bb = B_LOC
G4 = 4 * H  # 1024 gate width
TB = T * Bb  # 512 token-cols per h chunk

LAG = 16     # inter-layer lag in steps
SUB = 8      # xg-precompute granularity (steps per psum sub-chunk)
NSUB = T // SUB
W_TOT = T + 2 * LAG  # 288 waves

VC = 500           # FC vocab chunk (one PSUM bank)
NVC = V // VC      # 64
RBW = 2000         # readback/subtract chunk
NRB = V // RBW     # 16

FP16 = mybir.dt.float16
FP32 = mybir.dt.float32
AF = mybir.ActivationFunctionType
ALU = mybir.AluOpType
LAYER_DIMS = [E, H, H]


def mkap(tile_ap, off, dims):
    """Custom strided AP on a tile: off in elements, dims=[[step,count],...]."""
    return bass.AP(tile_ap.tensor, off,
                   [list(tile_ap.ap[0])] + [list(d) for d in dims])


def ksizes(dim):
    out = []
    while dim > 0:
        out.append(min(dim, 128))
        dim -= 128
    return out


def build_nc(has_lstm_bias=False, has_fc_bias=False):
    ntok = T * Bb  # 512 tokens per core
    n_mt = ntok // 128  # 4 fc token tiles

    nc = bacc.Bacc("TRN2", target_bir_lowering=False, debug=False,
                   num_devices=N_CORES)

    xids_d = nc.dram_tensor("xids", [ntok, 1], mybir.dt.int32, kind="ExternalInput")
    emb_d = nc.dram_tensor("emb", [V, E], FP32, kind="ExternalInput")
    wiT_d = [nc.dram_tensor(f"wiT{l}", [LAYER_DIMS[l], G4], FP16, kind="ExternalInput")
             for l in range(3)]
    whT_d = [nc.dram_tensor(f"whT{l}", [H, G4], FP16, kind="ExternalInput")
             for l in range(3)]
    bvec_d = [nc.dram_tensor(f"bvec{l}", [1, G4], FP16, kind="ExternalInput")
              for l in range(3)]
    fcWT_d = nc.dram_tensor("fcWT", [H, V], FP16, kind="ExternalInput")
    fcb_d = nc.dram_tensor("fcb", [1, V], FP16, kind="ExternalInput")
    out_d = nc.dram_tensor("out", [ntok, V], FP16, kind="ExternalOutput")

    with tile.TileContext(nc, num_cores=N_CORES) as tc:
        with (
            tc.tile_pool(name="state", bufs=1) as spool,
            tc.tile_pool(name="work", bufs=4) as work,
        ):
            # ---- persistent state ----
            hta = spool.tile([128, 6 * TB], FP16, tag="hta", name="hta")

            def ht_off(l, kc):
                return l * 2 * TB + kc * TB
            # cg: per layer 20 cols: [T16 (i,f,o,g tanh outputs) | c' (4)]
            cg = spool.tile([128, 60], FP32, tag="cg", name="cg")
            nc.vector.memset(cg[:], 0.0)
            zacc = spool.tile([128, 4 * NVC], FP32, tag="zacc", name="zacc")
            neglse = spool.tile([128, 4], FP32, tag="neglse", name="neglse")
            etrash = spool.tile([128, VC], FP16, tag="etrash", name="etrash")

            fcwpool = tc.alloc_tile_pool(name="fcw", bufs=1)
            fcw_sb = [fcwpool.tile([128, V], FP16, tag=f"fcw{kc}", name=f"fcw{kc}")
                      for kc in range(2)]
            fcb_sb = None
            if has_fc_bias:
                fcb_sb = fcwpool.tile([1, V], FP16, tag="fcb", name="fcb")

            stpool = tc.alloc_tile_pool(name="stage", bufs=4)
            rpool = tc.alloc_tile_pool(name="rb", bufs=4)
            wpool = tc.alloc_tile_pool(name="weights", bufs=1)
            # ---- Phase 0: LSTM weights to SBUF ----
            wiT_sb = []
            whT_sb = []
            bvec_sb = []
            for l in range(3):
                ks = ksizes(LAYER_DIMS[l])
                wi = wpool.tile([128, len(ks) * G4], FP16, tag=f"wiT{l}",
                                name=f"wiT{l}")
                for kc, ksz in enumerate(ks):
                    nc.sync.dma_start(
                        wi[0:ksz, kc * G4:(kc + 1) * G4],
                        wiT_d[l][kc * 128:kc * 128 + ksz, :],
                    )
                wiT_sb.append(wi)
                wh = wpool.tile([128, 2 * G4], FP16, tag=f"whT{l}", name=f"whT{l}")
                for kc in range(2):
                    nc.sync.dma_start(
                        wh[:, kc * G4:(kc + 1) * G4],
                        whT_d[l][kc * 128:(kc + 1) * 128, :],
                    )
                whT_sb.append(wh)
                if has_lstm_bias:
                    bv = wpool.tile([1, G4], FP16, tag=f"bvec{l}", name=f"bvec{l}")
                    nc.sync.dma_start(bv[:], bvec_d[l][:])
                    bvec_sb.append(bv)
                else:
                    bvec_sb.append(None)

            ones_sb = wpool.tile([1, 16], FP16, tag="ones", name="ones")
            nc.vector.memset(ones_sb[:], 1.0)
            onesf = spool.tile([1, 128], FP32, tag="onesf", name="onesf")
            nc.vector.memset(onesf[:], 1.0)
            ident = wpool.tile([128, 128], FP32, tag="ident", name="ident")
            make_identity(nc, ident[:])
            zrhs = wpool.tile([128, Bb], FP16, tag="zrhs", name="zrhs")
            nc.vector.memset(zrhs[:], 0.0)
            xT = wpool.tile([128, 2 * TB], FP16, tag="xT", name="xT")

            # ---- Phase 1: embedding gather + transpose into xT ----
            eks = ksizes(E)
            with tc.tile_pool(name="embps", bufs=2, space="PSUM") as eps:
                for gt in range(ntok // 128):
                    idt = work.tile([128, 1], mybir.dt.int32, tag="ids", name="ids")
                    nc.sync.dma_start(idt[:], xids_d[gt * 128:(gt + 1) * 128, :])
                    gat = work.tile([128, E], FP32, tag="gather", name="gather", bufs=2)
                    nc.gpsimd.indirect_dma_start(
                        out=gat[:],
                        out_offset=None,
                        in_=emb_d[:, :],
                        in_offset=bass.IndirectOffsetOnAxis(ap=idt[:, :1], axis=0),
                    )
                    for kc, ksz in enumerate(eks):
                        tp = eps.tile([128, 128], FP32, tag="tpsum", name="tpsum")
                        nc.tensor.transpose(
                            tp[0:ksz, 0:128],
                            gat[:, kc * 128:kc * 128 + ksz],
                            ident[:],
                        )
                        nc.vector.tensor_copy(
                            xT[0:ksz, kc * TB + gt * 128:kc * TB + (gt + 1) * 128],
                            tp[0:ksz, 0:128],
                        )

            # ---- Phase 2: wavefront with 2 chain streams + inline FC ----
            fcpp = tc.alloc_tile_pool(name="fcps", bufs=2, space="PSUM")
            gpp = tc.alloc_tile_pool(name="gps", bufs=1, space="PSUM")
            gp = gpp.tile([128, 3072], FP32, tag="gp", name="gp")
            openers = {}

            def emit_xg_piece(l, c, j):
                """Emit 2 of the 16 xg matmuls for (layer l, sub-chunk c)."""
                ks = ksizes(LAYER_DIMS[l])
                sb = (c % 2) * 1536 + l * 512
                for t in (2 * j, 2 * j + 1):
                    m, kc = t % 8, t // 8
                    ksz = ks[kc]
                    if l == 0:
                        rhs = xT[0:ksz, kc * TB + c * 16:kc * TB + (c + 1) * 16]
                    else:
                        rhs = hta[0:ksz,
                                  ht_off(l - 1, kc) + c * 16:ht_off(l - 1, kc) + (c + 1) * 16]
                    out = mkap(gp[:], sb + m * Bb, [[16, SUB], [1, Bb]])
                    is_open = (m == 0 and kc == 0)
                    mm = nc.tensor.matmul(
                        out,
                        lhsT=wiT_sb[l][0:ksz,
                                       kc * G4 + m * 128:kc * G4 + (m + 1) * 128],
                        rhs=rhs,
                        start=is_open,
                        stop=False,
                        skip_group_check=True,
                    )
                    if is_open:
                        openers[(l, c)] = mm.ins
                    else:
                        add_dep_helper(mm.ins, openers[(l, c)], sync=False,
                                       reason="slot opener order")
                if has_lstm_bias and j == 7:
                    for m in range(8):
                        mm = nc.tensor.matmul(
                            mkap(gp[:], sb + m * Bb, [[16, SUB], [1, Bb]]),
                            lhsT=bvec_sb[l][:, m * 128:(m + 1) * 128],
                            rhs=ones_sb[:, 0:16],
                            start=False,
                            stop=False,
                            skip_group_check=True,
                        )
                        add_dep_helper(mm.ins, openers[(l, c)], sync=False,
                                       reason="slot opener order")

            stage_state = {}

            def fc_mm(mt, v):
                """FC matmul for token-tile mt, vocab cols [v*VC, +VC) -> PSUM."""
                vs = v * VC
                ps = fcpp.tile([128, VC], FP32, tag="fcpsum", name="fcpsum")
                stage_state.setdefault("ps", {})[(mt, v)] = ps
                for kc in range(2):
                    nc.tensor.matmul(
                        ps[:],
                        lhsT=hta[:, ht_off(2, kc) + mt * 128:ht_off(2, kc) + (mt + 1) * 128],
                        rhs=fcw_sb[kc][:, vs:vs + VC],
                        start=(kc == 0),
                        stop=(kc == 1 and not has_fc_bias),
                        skip_group_check=True,
                    )
                if has_fc_bias:
                    nc.tensor.matmul(
                        ps[:],
                        lhsT=onesf[:, 0:128],
                        rhs=fcb_sb[:, vs:vs + VC],
                        start=False,
                        stop=True,
                        skip_group_check=True,
                    )
                return ps

            def fc_exp(mt, v, ps):
                """exp-accum directly from PSUM (fills the scalar wait gap)."""
                nc.scalar.activation(
                    etrash[:], ps[:], AF.Exp,
                    accum_out=zacc[:, mt * NVC + v:mt * NVC + v + 1],
                )

            def fc_stage(mt, v, ps):
                """fp32 PSUM -> fp16 stage (V); every 4th chunk DMA stage out."""
                if v % 4 == 0:
                    stage_state["tile"] = stpool.tile([128, 4 * VC], FP16,
                                                      tag="fcstage", name="fcstage")
                st = stage_state["tile"]
                nc.vector.tensor_copy(st[:, (v % 4) * VC:(v % 4 + 1) * VC], ps[:])
                if v % 4 == 3:
                    k = v // 4
                    nc.sync.dma_start(
                        out_d[mt * 128:(mt + 1) * 128, k * 4 * VC:(k + 1) * 4 * VC],
                        st[:],
                    )

            def fc_chunk(mt, v):
                ps = fc_mm(mt, v)
                fc_exp(mt, v, ps)
                fc_stage(mt, v, ps)

            def emit_lse(mt):
                """zacc[mt] -> neglse[:, mt].  Ln costs a table-switch pair."""
                zs = work.tile([128, 1], FP32, tag="zsum", name="zsum")
                nc.vector.tensor_reduce(
                    zs[:], zacc[:, mt * NVC:(mt + 1) * NVC],
                    op=ALU.add, axis=mybir.AxisListType.X,
                )
                lse = work.tile([128, 1], FP32, tag="lse", name="lse")
                nc.scalar.activation(lse[:], zs[:], AF.Ln)
                nc.vector.tensor_scalar_mul(neglse[:, mt:mt + 1], lse[:], -1.0)

            def emit_sub(mt, k):
                """Read back out_d chunk, add -lse, rewrite."""
                rb = rpool.tile([128, RBW], FP16, tag="rb", name="rb")
                nc.sync.dma_start(
                    rb[:], out_d[mt * 128:(mt + 1) * 128, k * RBW:(k + 1) * RBW])
                nc.vector.tensor_scalar_add(rb[:], rb[:], neglse[:, mt:mt + 1])
                nc.sync.dma_start(
                    out_d[mt * 128:(mt + 1) * 128, k * RBW:(k + 1) * RBW], rb[:])

            def emit_wh(group, w, gbase):
                for l in group:
                    tl = w - LAG * l
                    sb = gbase + l * 512
                    for kc in range(2):
                        if tl == 0:
                            rhs = zrhs[:, 0:Bb]
                        else:
                            rhs = hta[:, ht_off(l, kc) + (tl - 1) * Bb:
                                      ht_off(l, kc) + tl * Bb]
                        for m in range(8):
                            nc.tensor.matmul(
                                gp[:, sb + m * Bb:sb + (m + 1) * Bb],
                                lhsT=whT_sb[l][:, kc * G4 + m * 128:
                                               kc * G4 + (m + 1) * 128],
                                rhs=rhs,
                                start=False,
                                stop=(kc == 1),
                                skip_group_check=True,
                            )

            def chain_part1(group, w, gbase, tag):
                """tanh of gates, then fused cell update -> new c'."""
                l0, nl = group[0], len(group)
                nc.scalar.activation(
                    mkap(cg[:], 20 * l0, [[20, nl], [1, 16]]),
                    mkap(gp[:], gbase + l0 * 512, [[512, nl], [1, 16]]),
                    AF.Tanh,
                )
                prod = work.tile([128, 24], FP32, tag="prod" + tag,
                                 name="prod" + tag, bufs=4)
                # [p2,p1] = (T[i,f] + 1) * [T_g, c']
                nc.vector.scalar_tensor_tensor(
                    mkap(prod[:], 8 * l0, [[8, nl], [1, 8]]),
                    mkap(cg[:], 20 * l0, [[20, nl], [1, 8]]),
                    1.0,
                    mkap(cg[:], 20 * l0 + 12, [[20, nl], [1, 8]]),
                    ALU.add, ALU.mult,
                )
                # c'_new = 0.5*p1 + p2
                nc.vector.scalar_tensor_tensor(
                    mkap(cg[:], 20 * l0 + 16, [[20, nl], [1, 4]]),
                    mkap(prod[:], 8 * l0 + 4, [[8, nl], [1, 4]]),
                    0.5,
                    mkap(prod[:], 8 * l0, [[8, nl], [1, 4]]),
                    ALU.mult, ALU.add,
                )
                return prod

            def chain_part2(group, w, tag):
                """tanh(c) and h' = (T_o + 1) * tanh(c)."""
                l0, nl = group[0], len(group)
                tct = work.tile([128, 12], FP32, tag="tct" + tag,
                                name="tct" + tag, bufs=4)
                nc.scalar.activation(
                    mkap(tct[:], 4 * l0, [[4, nl], [1, 4]]),
                    mkap(cg[:], 20 * l0 + 16, [[20, nl], [1, 4]]),
                    AF.Tanh, scale=0.5,
                )
                for l in group:
                    nc.vector.scalar_tensor_tensor(
                        mkap(hta[:], 992 * l + w * Bb, [[TB, 2], [1, Bb]]),
                        cg[:, 20 * l + 8:20 * l + 12],
                        1.0,
                        tct[:, 4 * l:4 * l + 4],
                        ALU.add, ALU.mult,
                    )

            # xg calendar
            xg_cal = {}
            prologue_xg = []
            for l in range(3):
                for c in range(NSUB):
                    for j in range(8):
                        if l == 0:
                            w = SUB * (c - 1) + j
                        else:
                            w = LAG * l + SUB * c - 9 + j
                        if w < 0:
                            prologue_xg.append((l, c, j))
                        else:
                            xg_cal.setdefault(w, []).append((l, c, j))

            # FC calendar: token-tile mt ready at wave 64*mt+95
            fc_cal = {}
            for mt in range(3):
                for v in range(NVC):
                    fc_cal.setdefault(64 * mt + 96 + v, []).append((mt, v))
            # lse for mt0/mt1 inline, a few waves after the last chunk's exp
            lse_cal = {64 * mt + 96 + NVC + 4: mt for mt in range(2)}
            # subtract calendar for mt0/mt1
            sub_cal = {}
            for mt in range(2):
                for k in range(NRB):
                    sub_cal.setdefault(64 * mt + 96 + NVC + 7 + 2 * k, []).append((mt, k))

            for (l, c, j) in prologue_xg:
                emit_xg_piece(l, c, j)

            # fcW load: after the embedding/weight DMAs (first needed ~wave 96)
            for kc in range(2):
                for q in range(8):
                    nc.sync.dma_start(
                        fcw_sb[kc][:, q * 4000:(q + 1) * 4000],
                        fcWT_d[kc * 128:(kc + 1) * 128, q * 4000:(q + 1) * 4000])
            if has_fc_bias:
                nc.sync.dma_start(fcb_sb[:], fcb_d[:])

            for w in range(W_TOT):
                active = [l for l in range(3) if 0 <= w - LAG * l < T]
                g01 = [l for l in active if l < 2]
                g2 = [l for l in active if l == 2]
                P = (w // SUB) % 2
                s8 = w % SUB
                gbase = P * 1536 + s8 * 16

                emit_wh(g01, w, gbase)
                emit_wh(g2, w, gbase)
                if g01:
                    chain_part1(g01, w, gbase, "01")
                if g2:
                    chain_part1(g2, w, gbase, "2")
                fcs = fc_cal.get(w, ())
                for (mt, v) in fcs:
                    fc_mm(mt, v)
                if g01:
                    chain_part2(g01, w, "01")
                if g2:
                    chain_part2(g2, w, "2")
                # exp at the scalar-queue tail: fills the gap before the next
                # wave's first ACT (whose input matmuls are still running).
                # stage copies after the chain's V ops to avoid head-blocking.
                for (mt, v) in fcs:
                    fc_exp(mt, v, stage_state["ps"][(mt, v)])
                    fc_stage(mt, v, stage_state["ps"][(mt, v)])
                for (l, c, j) in xg_cal.get(w, ()):
                    emit_xg_piece(l, c, j)
                if w in lse_cal:
                    emit_lse(lse_cal[w])
                for (mt, k) in sub_cal.get(w, ()):
                    emit_sub(mt, k)

            wpool.release()
            gpp.release()

            # ---- Tail: mt2 subtract (overlaps mt3 FC), mt3 FC, lse3, subs
            emit_lse(2)
            for k in range(NRB):
                emit_sub(2, k)
            for v in range(NVC):
                fc_chunk(3, v)
            emit_lse(3)
            for k in range(NRB):
                emit_sub(3, k)

            rpool.release()
            stpool.release()
            fcpp.release()
            fcwpool.release()

    nc.compile()
    return nc


_nc_cache = {}


def _get_nc(has_lstm_bias, has_fc_bias):
    key = (has_lstm_bias, has_fc_bias)
    if key not in _nc_cache:
        _nc_cache[key] = build_nc(has_lstm_bias, has_fc_bias)
    return _nc_cache[key]


def prep_inputs(x, emb, Wi, Wh, bb, fcW, fcb):
    """Host-side shard + repack. Returns in_maps for the 8 cores.

    Gate rows reordered to [i,f,o,g].  Row scale 0.5 on i,f,o (sigmoid via
    tanh(x/2)); h-consuming weights additionally halved (h' = 2h); fcW halved.
    """
    perm = np.concatenate([np.arange(0, 512), np.arange(768, 1024),
                           np.arange(512, 768)])  # i,f | o | g
    rowscale = np.ones((G4, 1), np.float32)
    rowscale[0:768] = 0.5  # i,f,o rows: tanh(pre/2)
    shared = {
        "emb": np.ascontiguousarray(emb.astype(np.float32)),
        "fcWT": np.ascontiguousarray((fcW.T * 0.5).astype(np.float16)),
        "fcb": np.ascontiguousarray(fcb[None, :].astype(np.float16)),
    }
    for l in range(3):
        inscale = 1.0 if l == 0 else 0.5  # layers 1,2 consume h' = 2h
        shared[f"wiT{l}"] = np.ascontiguousarray(
            (Wi[l][perm] * rowscale * inscale).T.astype(np.float16))
        shared[f"whT{l}"] = np.ascontiguousarray(
            (Wh[l][perm] * rowscale * 0.5).T.astype(np.float16))
        shared[f"bvec{l}"] = np.ascontiguousarray(
            (bb[l][perm] * rowscale[:, 0])[None, :].astype(np.float16))
    in_maps = []
    for c in range(N_CORES):
        x_loc = x[c * B_LOC:(c + 1) * B_LOC, :]
        xids = np.ascontiguousarray(
            x_loc.T.reshape(-1, 1).astype(np.int32))  # [(t b), 1]
        m = dict(shared)
        m["xids"] = xids
        in_maps.append(m)
    return in_maps


def kernel(x, emb, Wi0, Wh0, b0, Wi1, Wh1, b1, Wi2, Wh2, b2, fcW, fcb,
           trace=False):
    x = np.asarray(x)
    bbs = [np.asarray(b0), np.asarray(b1), np.asarray(b2)]
    has_lstm_bias = bool(any(np.any(b) for b in bbs))
    has_fc_bias = bool(np.any(np.asarray(fcb)))
    nc = _get_nc(has_lstm_bias, has_fc_bias)
    in_maps = prep_inputs(
        np.asarray(x), np.asarray(emb),
        [np.asarray(Wi0), np.asarray(Wi1), np.asarray(Wi2)],
        [np.asarray(Wh0), np.asarray(Wh1), np.asarray(Wh2)],
        bbs, np.asarray(fcW), np.asarray(fcb))
    res = run_bass_kernel_spmd(nc, in_maps, core_ids=list(range(N_CORES)),
                               trace=trace)
    out = np.empty((B, T, V), np.float32)
    for c in range(N_CORES):
        oc = res.results[c]["out"].astype(np.float32).reshape(T, B_LOC, V)
        out[c * B_LOC:(c + 1) * B_LOC] = oc.transpose(1, 0, 2)
    kernel.last_results = res
    return out


# revision 25
# speedup vs baseline: 1.0504x; 1.0486x over previous
"""Trainium2 Bass kernel: 3-layer LSTM LM (embed -> 3xLSTM(H=256) -> FC 32000 -> log_softmax).

Strategy: data-parallel over batch across 8 cores (2 sequences per core).

v2 wave design:
- LSTM cell reformulated with tanh only: sigmoid(x) = (1+tanh(x/2))/2 with the
  1/2 folded into pre-scaled weights, and doubled states c' = 2c, h' = 2h
  (weights consuming h are pre-halved).  This keeps the whole recurrent chain
  in the `exp_and_others` ACT table set, so log-softmax exp-accumulation runs
  inline during the wavefront with no table switches.
- The per-wave nonlinear chain is split into two independent streams
  ({L0,L1} merged via strided APs, {L2}) so each stream's chain overlaps the
  other stream's matmuls of the next wave.
- Cell update uses fused scalar_tensor_tensor ops:
    T = tanh(gates)            (ACT, all 16 cols/layer; weights pre-scaled)
    [p2,p1] = (T[i,f] + 1) * [T_g, c']      (one strided STT)
    c'_new  = 0.5*p1 + p2                    (STT)
    tc      = tanh(0.5 * c'_new)             (ACT, free input scale)
    h'      = (T_o + 1) * tc                 (STT)
- FC logits: matmul -> PSUM; exp-accumulated directly from PSUM (scalar) and
  staged to DRAM fp16 via gpsimd cast-DMA (no stage copies).  lse for token
  tiles 0/1 computed inline (Ln costs a table-switch pair each); their
  subtract+rewrite also runs inline.  Tiles 2/3 finish in the tail.
"""

import sys

sys.path.insert(0, "/opt/trn_rl_repo")

import numpy as np

import concourse.bass as bass
import concourse.mybir as mybir
import concourse.tile as tile
from concourse import bacc
from concourse.bass_utils import run_bass_kernel_spmd
from concourse.masks import make_identity
from concourse.tile import add_dep_helper

# Problem dims
V = 32000
E = 200
H = 256
B = 16
T = 256
N_CORES = 8
B_LOC = B // N_CORES  # 2 sequences per core
Bb = B_LOC
G4 = 4 * H  # 1024 gate width
TB = T * Bb  # 512 token-cols per h chunk

LAG = 16     # inter-layer lag in steps
SUB = 8      # xg-precompute granularity (steps per psum sub-chunk)
NSUB = T // SUB
W_TOT = T + 2 * LAG  # 288 waves

VC = 500           # FC vocab chunk (one PSUM bank)
NVC = V // VC      # 64
RBW = 2000         # readback/subtract chunk
NRB = V // RBW     # 16

FP16 = mybir.dt.float16
FP32 = mybir.dt.float32
AF = mybir.ActivationFunctionType
ALU = mybir.AluOpType
LAYER_DIMS = [E, H, H]


def mkap(tile_ap, off, dims):
    """Custom strided AP on a tile: off in elements, dims=[[step,count],...]."""
    return bass.AP(tile_ap.tensor, off,
                   [list(tile_ap.ap[0])] + [list(d) for d in dims])


def ksizes(dim):
    out = []
    while dim > 0:
        out.append(min(dim, 128))
        dim -= 128
    return out


def build_nc(has_lstm_bias=False, has_fc_bias=False):
    ntok = T * Bb  # 512 tokens per core
    n_mt = ntok // 128  # 4 fc token tiles

    nc = bacc.Bacc("TRN2", target_bir_lowering=False, debug=False,
                   num_devices=N_CORES)

    xids_d = nc.dram_tensor("xids", [ntok, 1], mybir.dt.int32, kind="ExternalInput")
    emb_d = nc.dram_tensor("emb", [V, E], FP32, kind="ExternalInput")
    wiT_d = [nc.dram_tensor(f"wiT{l}", [LAYER_DIMS[l], G4], FP16, kind="ExternalInput")
             for l in range(3)]
    whT_d = [nc.dram_tensor(f"whT{l}", [H, G4], FP16, kind="ExternalInput")
             for l in range(3)]
    bvec_d = [nc.dram_tensor(f"bvec{l}", [1, G4], FP16, kind="ExternalInput")
              for l in range(3)]
    fcWT_d = nc.dram_tensor("fcWT", [H, V], FP16, kind="ExternalInput")
    fcb_d = nc.dram_tensor("fcb", [1, V], FP16, kind="ExternalInput")
    out_d = nc.dram_tensor("out", [ntok, V], FP16, kind="ExternalOutput")

    with tile.TileContext(nc, num_cores=N_CORES) as tc:
        with (
            tc.tile_pool(name="state", bufs=1) as spool,
            tc.tile_pool(name="work", bufs=4) as work,
        ):
            # ---- persistent state ----
            hta = spool.tile([128, 6 * TB], FP16, tag="hta", name="hta")

            def ht_off(l, kc):
                return l * 2 * TB + kc * TB
            # cg: per layer 20 cols: [T16 (i,f,o,g tanh outputs) | c' (4)]
            cg = spool.tile([128, 60], FP32, tag="cg", name="cg")
            nc.vector.memset(cg[:], 0.0)
            zacc = spool.tile([128, 4 * NRB], FP32, tag="zacc", name="zacc")
            neglse = spool.tile([128, 4], FP32, tag="neglse", name="neglse")
            etrash = spool.tile([128, 4 * VC], FP16, tag="etrash", name="etrash")

            fcwpool = tc.alloc_tile_pool(name="fcw", bufs=1)
            fcw_sb = [fcwpool.tile([128, V], FP16, tag=f"fcw{kc}", name=f"fcw{kc}")
                      for kc in range(2)]
            fcb_sb = None
            if has_fc_bias:
                fcb_sb = fcwpool.tile([1, V], FP16, tag="fcb", name="fcb")

            stpool = tc.alloc_tile_pool(name="stage", bufs=4)
            rpool = tc.alloc_tile_pool(name="rb", bufs=4)
            wpool = tc.alloc_tile_pool(name="weights", bufs=1)
            # ---- Phase 0: LSTM weights to SBUF ----
            wiT_sb = []
            whT_sb = []
            bvec_sb = []
            for l in range(3):
                ks = ksizes(LAYER_DIMS[l])
                wi = wpool.tile([128, len(ks) * G4], FP16, tag=f"wiT{l}",
                                name=f"wiT{l}")
                for kc, ksz in enumerate(ks):
                    nc.sync.dma_start(
                        wi[0:ksz, kc * G4:(kc + 1) * G4],
                        wiT_d[l][kc * 128:kc * 128 + ksz, :],
                    )
                wiT_sb.append(wi)
                wh = wpool.tile([128, 2 * G4], FP16, tag=f"whT{l}", name=f"whT{l}")
                for kc in range(2):
                    nc.sync.dma_start(
                        wh[:, kc * G4:(kc + 1) * G4],
                        whT_d[l][kc * 128:(kc + 1) * 128, :],
                    )
                whT_sb.append(wh)
                if has_lstm_bias:
                    bv = wpool.tile([1, G4], FP16, tag=f"bvec{l}", name=f"bvec{l}")
                    nc.sync.dma_start(bv[:], bvec_d[l][:])
                    bvec_sb.append(bv)
                else:
                    bvec_sb.append(None)

            ones_sb = wpool.tile([1, 16], FP16, tag="ones", name="ones")
            nc.vector.memset(ones_sb[:], 1.0)
            onesf = spool.tile([1, 128], FP32, tag="onesf", name="onesf")
            nc.vector.memset(onesf[:], 1.0)
            ident = wpool.tile([128, 128], FP32, tag="ident", name="ident")
            make_identity(nc, ident[:])
            zrhs = wpool.tile([128, Bb], FP16, tag="zrhs", name="zrhs")
            nc.vector.memset(zrhs[:], 0.0)
            xT = wpool.tile([128, 2 * TB], FP16, tag="xT", name="xT")

            # ---- Phase 1: embedding gather + transpose into xT ----
            eks = ksizes(E)
            with tc.tile_pool(name="embps", bufs=2, space="PSUM") as eps:
                for gt in range(ntok // 128):
                    idt = work.tile([128, 1], mybir.dt.int32, tag="ids", name="ids")
                    nc.sync.dma_start(idt[:], xids_d[gt * 128:(gt + 1) * 128, :])
                    gat = work.tile([128, E], FP32, tag="gather", name="gather", bufs=2)
                    nc.gpsimd.indirect_dma_start(
                        out=gat[:],
                        out_offset=None,
                        in_=emb_d[:, :],
                        in_offset=bass.IndirectOffsetOnAxis(ap=idt[:, :1], axis=0),
                    )
                    for kc, ksz in enumerate(eks):
                        tp = eps.tile([128, 128], FP32, tag="tpsum", name="tpsum")
                        nc.tensor.transpose(
                            tp[0:ksz, 0:128],
                            gat[:, kc * 128:kc * 128 + ksz],
                            ident[:],
                        )
                        nc.vector.tensor_copy(
                            xT[0:ksz, kc * TB + gt * 128:kc * TB + (gt + 1) * 128],
                            tp[0:ksz, 0:128],
                        )

            # ---- Phase 2: wavefront with 2 chain streams + inline FC ----
            fcpp = tc.alloc_tile_pool(name="fcps", bufs=2, space="PSUM")
            gpp = tc.alloc_tile_pool(name="gps", bufs=1, space="PSUM")
            gp = gpp.tile([128, 3072], FP32, tag="gp", name="gp")
            openers = {}

            def emit_xg_piece(l, c, j):
                """Emit 2 of the 16 xg matmuls for (layer l, sub-chunk c)."""
                ks = ksizes(LAYER_DIMS[l])
                sb = (c % 2) * 1536 + l * 512
                for t in (2 * j, 2 * j + 1):
                    m, kc = t % 8, t // 8
                    ksz = ks[kc]
                    if l == 0:
                        rhs = xT[0:ksz, kc * TB + c * 16:kc * TB + (c + 1) * 16]
                    else:
                        rhs = hta[0:ksz,
                                  ht_off(l - 1, kc) + c * 16:ht_off(l - 1, kc) + (c + 1) * 16]
                    out = mkap(gp[:], sb + m * Bb, [[16, SUB], [1, Bb]])
                    is_open = (m == 0 and kc == 0)
                    mm = nc.tensor.matmul(
                        out,
                        lhsT=wiT_sb[l][0:ksz,
                                       kc * G4 + m * 128:kc * G4 + (m + 1) * 128],
                        rhs=rhs,
                        start=is_open,
                        stop=False,
                        skip_group_check=True,
                    )
                    if is_open:
                        openers[(l, c)] = mm.ins
                    else:
                        add_dep_helper(mm.ins, openers[(l, c)], sync=False,
                                       reason="slot opener order")
                if has_lstm_bias and j == 7:
                    for m in range(8):
                        mm = nc.tensor.matmul(
                            mkap(gp[:], sb + m * Bb, [[16, SUB], [1, Bb]]),
                            lhsT=bvec_sb[l][:, m * 128:(m + 1) * 128],
                            rhs=ones_sb[:, 0:16],
                            start=False,
                            stop=False,
                            skip_group_check=True,
                        )
                        add_dep_helper(mm.ins, openers[(l, c)], sync=False,
                                       reason="slot opener order")

            stage_state = {}

            def fc_mm(mt, v):
                """FC matmul for token-tile mt, vocab cols [v*VC, +VC) -> PSUM."""
                vs = v * VC
                ps = fcpp.tile([128, VC], FP32, tag="fcpsum", name="fcpsum")
                stage_state.setdefault("ps", {})[(mt, v)] = ps
                for kc in range(2):
                    nc.tensor.matmul(
                        ps[:],
                        lhsT=hta[:, ht_off(2, kc) + mt * 128:ht_off(2, kc) + (mt + 1) * 128],
                        rhs=fcw_sb[kc][:, vs:vs + VC],
                        start=(kc == 0),
                        stop=(kc == 1 and not has_fc_bias),
                        skip_group_check=True,
                    )
                if has_fc_bias:
                    nc.tensor.matmul(
                        ps[:],
                        lhsT=onesf[:, 0:128],
                        rhs=fcb_sb[:, vs:vs + VC],
                        start=False,
                        stop=True,
                        skip_group_check=True,
                    )
                return ps

            def fc_stage(mt, v, ps):
                """fp32 PSUM -> fp16 stage (V); every 4th chunk DMA stage out
                and exp-accumulate the 2000-col stage in one scalar op."""
                if v % 4 == 0:
                    stage_state["tile"] = stpool.tile([128, 4 * VC], FP16,
                                                      tag="fcstage", name="fcstage")
                st = stage_state["tile"]
                nc.vector.tensor_copy(st[:, (v % 4) * VC:(v % 4 + 1) * VC], ps[:])
                if v % 4 == 3:
                    k = v // 4
                    dout = nc.sync.dma_start(
                        out_d[mt * 128:(mt + 1) * 128, k * 4 * VC:(k + 1) * 4 * VC],
                        st[:],
                    )
                    stage_state.setdefault("out", {})[(mt, k)] = dout.ins
                    nc.scalar.activation(
                        etrash[:], st[:], AF.Exp,
                        accum_out=zacc[:, mt * NRB + k:mt * NRB + k + 1],
                    )

            def fc_chunk(mt, v):
                ps = fc_mm(mt, v)
                fc_stage(mt, v, ps)

            def emit_lse(mt):
                """zacc[mt] -> neglse[:, mt].  Ln costs a table-switch pair."""
                zs = work.tile([128, 1], FP32, tag="zsum", name="zsum")
                nc.vector.tensor_reduce(
                    zs[:], zacc[:, mt * NRB:(mt + 1) * NRB],
                    op=ALU.add, axis=mybir.AxisListType.X,
                )
                lse = work.tile([128, 1], FP32, tag="lse", name="lse")
                nc.scalar.activation(lse[:], zs[:], AF.Ln)
                nc.vector.tensor_scalar_mul(neglse[:, mt:mt + 1], lse[:], -1.0)

            def emit_sub(mt, k):
                """Read back out_d chunk, add -lse, rewrite."""
                rb = rpool.tile([128, RBW], FP16, tag="rb", name="rb")
                din = nc.sync.dma_start(
                    rb[:], out_d[mt * 128:(mt + 1) * 128, k * RBW:(k + 1) * RBW])
                src = stage_state.get("out", {}).get((mt, k))
                if src is not None:
                    add_dep_helper(din.ins, src, sync=True,
                                   reason="out_d staged-write before readback")
                nc.vector.tensor_scalar_add(rb[:], rb[:], neglse[:, mt:mt + 1])
                nc.sync.dma_start(
                    out_d[mt * 128:(mt + 1) * 128, k * RBW:(k + 1) * RBW], rb[:])

            def emit_wh(group, w, gbase):
                for l in group:
                    tl = w - LAG * l
                    sb = gbase + l * 512
                    for kc in range(2):
                        if tl == 0:
                            rhs = zrhs[:, 0:Bb]
                        else:
                            rhs = hta[:, ht_off(l, kc) + (tl - 1) * Bb:
                                      ht_off(l, kc) + tl * Bb]
                        for m in range(8):
                            nc.tensor.matmul(
                                gp[:, sb + m * Bb:sb + (m + 1) * Bb],
                                lhsT=whT_sb[l][:, kc * G4 + m * 128:
                                               kc * G4 + (m + 1) * 128],
                                rhs=rhs,
                                start=False,
                                stop=(kc == 1),
                                skip_group_check=True,
                            )

            def chain_part1(group, w, gbase, tag):
                """tanh of gates, then fused cell update -> new c'."""
                l0, nl = group[0], len(group)
                nc.scalar.activation(
                    mkap(cg[:], 20 * l0, [[20, nl], [1, 16]]),
                    mkap(gp[:], gbase + l0 * 512, [[512, nl], [1, 16]]),
                    AF.Tanh,
                )
                prod = work.tile([128, 24], FP32, tag="prod" + tag,
                                 name="prod" + tag, bufs=4)
                # [p2,p1] = (T[i,f] + 1) * [T_g, c']
                nc.vector.scalar_tensor_tensor(
                    mkap(prod[:], 8 * l0, [[8, nl], [1, 8]]),
                    mkap(cg[:], 20 * l0, [[20, nl], [1, 8]]),
                    1.0,
                    mkap(cg[:], 20 * l0 + 12, [[20, nl], [1, 8]]),
                    ALU.add, ALU.mult,
                )
                # c'_new = 0.5*p1 + p2
                nc.vector.scalar_tensor_tensor(
                    mkap(cg[:], 20 * l0 + 16, [[20, nl], [1, 4]]),
                    mkap(prod[:], 8 * l0 + 4, [[8, nl], [1, 4]]),
                    0.5,
                    mkap(prod[:], 8 * l0, [[8, nl], [1, 4]]),
                    ALU.mult, ALU.add,
                )
                return prod

            def chain_part2(group, w, tag):
                """tanh(c) and h' = (T_o + 1) * tanh(c)."""
                l0, nl = group[0], len(group)
                tct = work.tile([128, 12], FP32, tag="tct" + tag,
                                name="tct" + tag, bufs=4)
                nc.scalar.activation(
                    mkap(tct[:], 4 * l0, [[4, nl], [1, 4]]),
                    mkap(cg[:], 20 * l0 + 16, [[20, nl], [1, 4]]),
                    AF.Tanh, scale=0.5,
                )
                for l in group:
                    nc.vector.scalar_tensor_tensor(
                        mkap(hta[:], 992 * l + w * Bb, [[TB, 2], [1, Bb]]),
                        cg[:, 20 * l + 8:20 * l + 12],
                        1.0,
                        tct[:, 4 * l:4 * l + 4],
                        ALU.add, ALU.mult,
                    )

            # xg calendar
            xg_cal = {}
            prologue_xg = []
            for l in range(3):
                for c in range(NSUB):
                    for j in range(8):
                        if l == 0:
                            w = SUB * (c - 1) + j
                        else:
                            w = LAG * l + SUB * c - 9 + j
                        if w < 0:
                            prologue_xg.append((l, c, j))
                        else:
                            xg_cal.setdefault(w, []).append((l, c, j))

            # FC calendar: token-tile mt ready at wave 64*mt+95
            fc_cal = {}
            for mt in range(3):
                for v in range(NVC):
                    fc_cal.setdefault(64 * mt + 96 + v, []).append((mt, v))
            # lse for mt0/mt1 inline, a few waves after the last chunk's exp
            lse_cal = {64 * mt + 96 + NVC + 4: mt for mt in range(2)}
            # subtract calendar for mt0/mt1
            sub_cal = {}
            for mt in range(2):
                for k in range(NRB):
                    sub_cal.setdefault(64 * mt + 96 + NVC + 7 + 2 * k, []).append((mt, k))

            for (l, c, j) in prologue_xg:
                emit_xg_piece(l, c, j)

            # fcW load: after the embedding/weight DMAs (first needed ~wave 96)
            for kc in range(2):
                for q in range(8):
                    nc.sync.dma_start(
                        fcw_sb[kc][:, q * 4000:(q + 1) * 4000],
                        fcWT_d[kc * 128:(kc + 1) * 128, q * 4000:(q + 1) * 4000])
            if has_fc_bias:
                nc.sync.dma_start(fcb_sb[:], fcb_d[:])

            for w in range(W_TOT):
                active = [l for l in range(3) if 0 <= w - LAG * l < T]
                g01 = [l for l in active if l < 2]
                g2 = [l for l in active if l == 2]
                P = (w // SUB) % 2
                s8 = w % SUB
                gbase = P * 1536 + s8 * 16

                emit_wh(g01, w, gbase)
                emit_wh(g2, w, gbase)
                if g01:
                    chain_part1(g01, w, gbase, "01")
                if g2:
                    chain_part1(g2, w, gbase, "2")
                fcs = fc_cal.get(w, ())
                for (mt, v) in fcs:
                    fc_mm(mt, v)
                if g01:
                    chain_part2(g01, w, "01")
                if g2:
                    chain_part2(g2, w, "2")
                # stage copies after the chain's V ops to avoid head-blocking;
                # the 2000-col exp lands at the scalar-queue tail every 4th wave.
                for (mt, v) in fcs:
                    fc_stage(mt, v, stage_state["ps"][(mt, v)])
                for (l, c, j) in xg_cal.get(w, ()):
                    emit_xg_piece(l, c, j)
                if w in lse_cal:
                    emit_lse(lse_cal[w])
                for (mt, k) in sub_cal.get(w, ()):
                    emit_sub(mt, k)

            wpool.release()
            gpp.release()

            # ---- Tail: mt2 subtract (overlaps mt3 FC), mt3 FC, lse3, subs
            emit_lse(2)
            for k in range(NRB):
                emit_sub(2, k)
            for v in range(NVC):
                fc_chunk(3, v)
            emit_lse(3)
            for k in range(NRB):
                emit_sub(3, k)

            rpool.release()
            stpool.release()
            fcpp.release()
            fcwpool.release()

    nc.compile()
    return nc


_nc_cache = {}


def _get_nc(has_lstm_bias, has_fc_bias):
    key = (has_lstm_bias, has_fc_bias)
    if key not in _nc_cache:
        _nc_cache[key] = build_nc(has_lstm_bias, has_fc_bias)
    return _nc_cache[key]


def prep_inputs(x, emb, Wi, Wh, bb, fcW, fcb):
    """Host-side shard + repack. Returns in_maps for the 8 cores.

    Gate rows reordered to [i,f,o,g].  Row scale 0.5 on i,f,o (sigmoid via
    tanh(x/2)); h-consuming weights additionally halved (h' = 2h); fcW halved.
    """
    perm = np.concatenate([np.arange(0, 512), np.arange(768, 1024),
                           np.arange(512, 768)])  # i,f | o | g
    rowscale = np.ones((G4, 1), np.float32)
    rowscale[0:768] = 0.5  # i,f,o rows: tanh(pre/2)
    shared = {
        "emb": np.ascontiguousarray(emb.astype(np.float32)),
        "fcWT": np.ascontiguousarray((fcW.T * 0.5).astype(np.float16)),
        "fcb": np.ascontiguousarray(fcb[None, :].astype(np.float16)),
    }
    for l in range(3):
        inscale = 1.0 if l == 0 else 0.5  # layers 1,2 consume h' = 2h
        shared[f"wiT{l}"] = np.ascontiguousarray(
            (Wi[l][perm] * rowscale * inscale).T.astype(np.float16))
        shared[f"whT{l}"] = np.ascontiguousarray(
            (Wh[l][perm] * rowscale * 0.5).T.astype(np.float16))
        shared[f"bvec{l}"] = np.ascontiguousarray(
            (bb[l][perm] * rowscale[:, 0])[None, :].astype(np.float16))
    in_maps = []
    for c in range(N_CORES):
        x_loc = x[c * B_LOC:(c + 1) * B_LOC, :]
        xids = np.ascontiguousarray(
            x_loc.T.reshape(-1, 1).astype(np.int32))  # [(t b), 1]
        m = dict(shared)
        m["xids"] = xids
        in_maps.append(m)
    return in_maps


def kernel(x, emb, Wi0, Wh0, b0, Wi1, Wh1, b1, Wi2, Wh2, b2, fcW, fcb,
           trace=False):
    x = np.asarray(x)
    bbs = [np.asarray(b0), np.asarray(b1), np.asarray(b2)]
    has_lstm_bias = bool(any(np.any(b) for b in bbs))
    has_fc_bias = bool(np.any(np.asarray(fcb)))
    nc = _get_nc(has_lstm_bias, has_fc_bias)
    in_maps = prep_inputs(
        np.asarray(x), np.asarray(emb),
        [np.asarray(Wi0), np.asarray(Wi1), np.asarray(Wi2)],
        [np.asarray(Wh0), np.asarray(Wh1), np.asarray(Wh2)],
        bbs, np.asarray(fcW), np.asarray(fcb))
    res = run_bass_kernel_spmd(nc, in_maps, core_ids=list(range(N_CORES)),
                               trace=trace)
    out = np.empty((B, T, V), np.float32)
    for c in range(N_CORES):
        oc = res.results[c]["out"].astype(np.float32).reshape(T, B_LOC, V)
        out[c * B_LOC:(c + 1) * B_LOC] = oc.transpose(1, 0, 2)
    kernel.last_results = res
    return out


# revision 29
# speedup vs baseline: 1.0515x; 1.0010x over previous
"""Trainium2 Bass kernel: 3-layer LSTM LM (embed -> 3xLSTM(H=256) -> FC 32000 -> log_softmax).

Strategy: data-parallel over batch across 8 cores (2 sequences per core).

v2 wave design:
- LSTM cell reformulated with tanh only: sigmoid(x) = (1+tanh(x/2))/2 with the
  1/2 folded into pre-scaled weights, and doubled states c' = 2c, h' = 2h
  (weights consuming h are pre-halved).  This keeps the whole recurrent chain
  in the `exp_and_others` ACT table set, so log-softmax exp-accumulation runs
  inline during the wavefront with no table switches.
- The per-wave nonlinear chain is split into two independent streams
  ({L0,L1} merged via strided APs, {L2}) so each stream's chain overlaps the
  other stream's matmuls of the next wave.
- Cell update uses fused scalar_tensor_tensor ops:
    T = tanh(gates)            (ACT, all 16 cols/layer; weights pre-scaled)
    [p2,p1] = (T[i,f] + 1) * [T_g, c']      (one strided STT)
    c'_new  = 0.5*p1 + p2                    (STT)
    tc      = tanh(0.5 * c'_new)             (ACT, free input scale)
    h'      = (T_o + 1) * tc                 (STT)
- FC logits: matmul -> PSUM; exp-accumulated directly from PSUM (scalar) and
  staged to DRAM fp16 via gpsimd cast-DMA (no stage copies).  lse for token
  tiles 0/1 computed inline (Ln costs a table-switch pair each); their
  subtract+rewrite also runs inline.  Tiles 2/3 finish in the tail.
"""

import sys

sys.path.insert(0, "/opt/trn_rl_repo")

import numpy as np

import concourse.bass as bass
import concourse.mybir as mybir
import concourse.tile as tile
from concourse import bacc
from concourse.bass_utils import run_bass_kernel_spmd
from concourse.masks import make_identity
from concourse.tile import add_dep_helper

# Problem dims
V = 32000
E = 200
H = 256
B = 16
T = 256
N_CORES = 8
B_LOC = B // N_CORES  # 2 sequences per core
Bb = B_LOC
G4 = 4 * H  # 1024 gate width
TB = T * Bb  # 512 token-cols per h chunk

LAG = 16     # inter-layer lag in steps
SUB = 8      # xg-precompute granularity (steps per psum sub-chunk)
NSUB = T // SUB
W_TOT = T + 2 * LAG  # 288 waves

VC = 500           # FC vocab chunk (one PSUM bank)
NVC = V // VC      # 64
RBW = 2000         # readback/subtract chunk
NRB = V // RBW     # 16

FP16 = mybir.dt.float16
FP32 = mybir.dt.float32
AF = mybir.ActivationFunctionType
ALU = mybir.AluOpType
LAYER_DIMS = [E, H, H]


def mkap(tile_ap, off, dims):
    """Custom strided AP on a tile: off in elements, dims=[[step,count],...]."""
    return bass.AP(tile_ap.tensor, off,
                   [list(tile_ap.ap[0])] + [list(d) for d in dims])


def ksizes(dim):
    out = []
    while dim > 0:
        out.append(min(dim, 128))
        dim -= 128
    return out


def build_nc(has_lstm_bias=False, has_fc_bias=False):
    ntok = T * Bb  # 512 tokens per core
    n_mt = ntok // 128  # 4 fc token tiles

    nc = bacc.Bacc("TRN2", target_bir_lowering=False, debug=False,
                   num_devices=N_CORES)

    xids_d = nc.dram_tensor("xids", [ntok, 1], mybir.dt.int32, kind="ExternalInput")
    emb_d = nc.dram_tensor("emb", [V, E], FP32, kind="ExternalInput")
    wiT_d = [nc.dram_tensor(f"wiT{l}", [LAYER_DIMS[l], G4], FP16, kind="ExternalInput")
             for l in range(3)]
    whT_d = [nc.dram_tensor(f"whT{l}", [H, G4], FP16, kind="ExternalInput")
             for l in range(3)]
    bvec_d = [nc.dram_tensor(f"bvec{l}", [1, G4], FP16, kind="ExternalInput")
              for l in range(3)]
    fcWT_d = nc.dram_tensor("fcWT", [H, V], FP16, kind="ExternalInput")
    fcb_d = nc.dram_tensor("fcb", [1, V], FP16, kind="ExternalInput")
    out_d = nc.dram_tensor("out", [ntok, V], FP16, kind="ExternalOutput")

    with tile.TileContext(nc, num_cores=N_CORES) as tc:
        with (
            tc.tile_pool(name="state", bufs=1) as spool,
            tc.tile_pool(name="work", bufs=4) as work,
        ):
            # ---- persistent state ----
            hta = spool.tile([128, 6 * TB], FP16, tag="hta", name="hta")

            def ht_off(l, kc):
                return l * 2 * TB + kc * TB
            # cg: per layer 20 cols: [T16 (i,f,o,g tanh outputs) | c' (4)]
            cg = spool.tile([128, 60], FP32, tag="cg", name="cg")
            nc.vector.memset(cg[:], 0.0)
            zacc = spool.tile([128, 4 * NRB], FP32, tag="zacc", name="zacc")
            neglse = spool.tile([128, 4], FP32, tag="neglse", name="neglse")
            etrash = spool.tile([128, 4 * VC], FP16, tag="etrash", name="etrash")

            fcwpool = tc.alloc_tile_pool(name="fcw", bufs=1)
            fcw_sb = [fcwpool.tile([128, V], FP16, tag=f"fcw{kc}", name=f"fcw{kc}")
                      for kc in range(2)]
            fcb_sb = None
            if has_fc_bias:
                fcb_sb = fcwpool.tile([1, V], FP16, tag="fcb", name="fcb")

            stpool = tc.alloc_tile_pool(name="stage", bufs=4)
            rpool = tc.alloc_tile_pool(name="rb", bufs=4)
            wpool = tc.alloc_tile_pool(name="weights", bufs=1)
            # ---- Phase 0: LSTM weights to SBUF ----
            wiT_sb = []
            whT_sb = []
            bvec_sb = []
            for l in range(3):
                ks = ksizes(LAYER_DIMS[l])
                wi = wpool.tile([128, len(ks) * G4], FP16, tag=f"wiT{l}",
                                name=f"wiT{l}")
                for kc, ksz in enumerate(ks):
                    nc.sync.dma_start(
                        wi[0:ksz, kc * G4:(kc + 1) * G4],
                        wiT_d[l][kc * 128:kc * 128 + ksz, :],
                    )
                wiT_sb.append(wi)
                wh = wpool.tile([128, 2 * G4], FP16, tag=f"whT{l}", name=f"whT{l}")
                for kc in range(2):
                    nc.sync.dma_start(
                        wh[:, kc * G4:(kc + 1) * G4],
                        whT_d[l][kc * 128:(kc + 1) * 128, :],
                    )
                whT_sb.append(wh)
                if has_lstm_bias:
                    bv = wpool.tile([1, G4], FP16, tag=f"bvec{l}", name=f"bvec{l}")
                    nc.sync.dma_start(bv[:], bvec_d[l][:])
                    bvec_sb.append(bv)
                else:
                    bvec_sb.append(None)

            ones_sb = wpool.tile([1, 16], FP16, tag="ones", name="ones")
            nc.vector.memset(ones_sb[:], 1.0)
            onesf = spool.tile([1, 128], FP32, tag="onesf", name="onesf")
            nc.vector.memset(onesf[:], 1.0)
            ident = wpool.tile([128, 128], FP32, tag="ident", name="ident")
            make_identity(nc, ident[:])
            zrhs = wpool.tile([128, Bb], FP16, tag="zrhs", name="zrhs")
            nc.vector.memset(zrhs[:], 0.0)
            xT = wpool.tile([128, 2 * TB], FP16, tag="xT", name="xT")

            # ---- Phase 1: embedding gather + transpose into xT ----
            eks = ksizes(E)
            with tc.tile_pool(name="embps", bufs=2, space="PSUM") as eps:
                for gt in range(ntok // 128):
                    idt = work.tile([128, 1], mybir.dt.int32, tag="ids", name="ids")
                    nc.sync.dma_start(idt[:], xids_d[gt * 128:(gt + 1) * 128, :])
                    gat = work.tile([128, E], FP32, tag="gather", name="gather", bufs=2)
                    nc.gpsimd.indirect_dma_start(
                        out=gat[:],
                        out_offset=None,
                        in_=emb_d[:, :],
                        in_offset=bass.IndirectOffsetOnAxis(ap=idt[:, :1], axis=0),
                    )
                    for kc, ksz in enumerate(eks):
                        tp = eps.tile([128, 128], FP32, tag="tpsum", name="tpsum")
                        nc.tensor.transpose(
                            tp[0:ksz, 0:128],
                            gat[:, kc * 128:kc * 128 + ksz],
                            ident[:],
                        )
                        nc.vector.tensor_copy(
                            xT[0:ksz, kc * TB + gt * 128:kc * TB + (gt + 1) * 128],
                            tp[0:ksz, 0:128],
                        )

            # ---- Phase 2: wavefront with 2 chain streams + inline FC ----
            fcpp = tc.alloc_tile_pool(name="fcps", bufs=2, space="PSUM")
            gpp = tc.alloc_tile_pool(name="gps", bufs=1, space="PSUM")
            gp = gpp.tile([128, 3072], FP32, tag="gp", name="gp")
            openers = {}

            def emit_xg_piece(l, c, j):
                """Emit 2 of the 16 xg matmuls for (layer l, sub-chunk c)."""
                ks = ksizes(LAYER_DIMS[l])
                sb = (c % 2) * 1536 + l * 512
                for t in (2 * j, 2 * j + 1):
                    m, kc = t % 8, t // 8
                    ksz = ks[kc]
                    if l == 0:
                        rhs = xT[0:ksz, kc * TB + c * 16:kc * TB + (c + 1) * 16]
                    else:
                        rhs = hta[0:ksz,
                                  ht_off(l - 1, kc) + c * 16:ht_off(l - 1, kc) + (c + 1) * 16]
                    out = mkap(gp[:], sb + m * Bb, [[16, SUB], [1, Bb]])
                    is_open = (m == 0 and kc == 0)
                    mm = nc.tensor.matmul(
                        out,
                        lhsT=wiT_sb[l][0:ksz,
                                       kc * G4 + m * 128:kc * G4 + (m + 1) * 128],
                        rhs=rhs,
                        start=is_open,
                        stop=False,
                        skip_group_check=True,
                    )
                    if is_open:
                        openers[(l, c)] = mm.ins
                    else:
                        add_dep_helper(mm.ins, openers[(l, c)], sync=False,
                                       reason="slot opener order")
                if has_lstm_bias and j == 7:
                    for m in range(8):
                        mm = nc.tensor.matmul(
                            mkap(gp[:], sb + m * Bb, [[16, SUB], [1, Bb]]),
                            lhsT=bvec_sb[l][:, m * 128:(m + 1) * 128],
                            rhs=ones_sb[:, 0:16],
                            start=False,
                            stop=False,
                            skip_group_check=True,
                        )
                        add_dep_helper(mm.ins, openers[(l, c)], sync=False,
                                       reason="slot opener order")

            stage_state = {}

            def fc_mm(mt, v):
                """FC matmul for token-tile mt, vocab cols [v*VC, +VC) -> PSUM."""
                vs = v * VC
                ps = fcpp.tile([128, VC], FP32, tag="fcpsum", name="fcpsum")
                stage_state.setdefault("ps", {})[(mt, v)] = ps
                for kc in range(2):
                    nc.tensor.matmul(
                        ps[:],
                        lhsT=hta[:, ht_off(2, kc) + mt * 128:ht_off(2, kc) + (mt + 1) * 128],
                        rhs=fcw_sb[kc][:, vs:vs + VC],
                        start=(kc == 0),
                        stop=(kc == 1 and not has_fc_bias),
                        skip_group_check=True,
                    )
                if has_fc_bias:
                    nc.tensor.matmul(
                        ps[:],
                        lhsT=onesf[:, 0:128],
                        rhs=fcb_sb[:, vs:vs + VC],
                        start=False,
                        stop=True,
                        skip_group_check=True,
                    )
                return ps

            def fc_copy(mt, v, ps):
                """fp32 PSUM -> fp16 stage slice on Vector."""
                if v % 4 == 0:
                    stage_state["tile"] = stpool.tile([128, 4 * VC], FP16,
                                                      tag="fcstage", name="fcstage")
                st = stage_state["tile"]
                nc.vector.tensor_copy(st[:, (v % 4) * VC:(v % 4 + 1) * VC], ps[:])

            def fc_flush(mt, v):
                """Every 4th chunk: DMA the 2000-col stage out and
                exp-accumulate it in one scalar op (at the scalar-queue tail)."""
                if v % 4 != 3:
                    return
                st = stage_state["tile"]
                k = v // 4
                dout = nc.sync.dma_start(
                    out_d[mt * 128:(mt + 1) * 128, k * 4 * VC:(k + 1) * 4 * VC],
                    st[:],
                )
                stage_state.setdefault("out", {})[(mt, k)] = dout.ins
                nc.scalar.activation(
                    etrash[:], st[:], AF.Exp,
                    accum_out=zacc[:, mt * NRB + k:mt * NRB + k + 1],
                )

            def fc_chunk(mt, v):
                ps = fc_mm(mt, v)
                fc_copy(mt, v, ps)
                fc_flush(mt, v)

            def emit_lse(mt):
                """zacc[mt] -> neglse[:, mt].  Ln costs a table-switch pair."""
                zs = work.tile([128, 1], FP32, tag="zsum", name="zsum")
                nc.vector.tensor_reduce(
                    zs[:], zacc[:, mt * NRB:(mt + 1) * NRB],
                    op=ALU.add, axis=mybir.AxisListType.X,
                )
                lse = work.tile([128, 1], FP32, tag="lse", name="lse")
                nc.scalar.activation(lse[:], zs[:], AF.Ln)
                nc.vector.tensor_scalar_mul(neglse[:, mt:mt + 1], lse[:], -1.0)

            def emit_sub(mt, k):
                """Read back out_d chunk, add -lse, rewrite."""
                rb = rpool.tile([128, RBW], FP16, tag="rb", name="rb")
                din = nc.sync.dma_start(
                    rb[:], out_d[mt * 128:(mt + 1) * 128, k * RBW:(k + 1) * RBW])
                src = stage_state.get("out", {}).get((mt, k))
                if src is not None:
                    add_dep_helper(din.ins, src, sync=True,
                                   reason="out_d staged-write before readback")
                nc.vector.tensor_scalar_add(rb[:], rb[:], neglse[:, mt:mt + 1])
                nc.sync.dma_start(
                    out_d[mt * 128:(mt + 1) * 128, k * RBW:(k + 1) * RBW], rb[:])

            def emit_wh(group, w, gbase):
                # kc-major so the kc0 matmuls depend only on the h' chunk-0
                # write of the previous wave (which lands first).
                for kc in range(2):
                    for l in group:
                        tl = w - LAG * l
                        sb = gbase + l * 512
                        if tl == 0:
                            rhs = zrhs[:, 0:Bb]
                        else:
                            rhs = hta[:, ht_off(l, kc) + (tl - 1) * Bb:
                                      ht_off(l, kc) + tl * Bb]
                        for m in range(8):
                            nc.tensor.matmul(
                                gp[:, sb + m * Bb:sb + (m + 1) * Bb],
                                lhsT=whT_sb[l][:, kc * G4 + m * 128:
                                               kc * G4 + (m + 1) * 128],
                                rhs=rhs,
                                start=False,
                                stop=(kc == 1),
                                skip_group_check=True,
                            )

            def chain_part1(group, w, gbase, tag):
                """tanh of gates, then fused cell update -> new c'."""
                l0, nl = group[0], len(group)
                nc.scalar.activation(
                    mkap(cg[:], 20 * l0, [[20, nl], [1, 16]]),
                    mkap(gp[:], gbase + l0 * 512, [[512, nl], [1, 16]]),
                    AF.Tanh,
                )
                prod = work.tile([128, 24], FP32, tag="prod" + tag,
                                 name="prod" + tag, bufs=4)
                # [p2,p1] = (T[i,f] + 1) * [T_g, c']
                nc.vector.scalar_tensor_tensor(
                    mkap(prod[:], 8 * l0, [[8, nl], [1, 8]]),
                    mkap(cg[:], 20 * l0, [[20, nl], [1, 8]]),
                    1.0,
                    mkap(cg[:], 20 * l0 + 12, [[20, nl], [1, 8]]),
                    ALU.add, ALU.mult,
                )
                # c'_new = 0.5*p1 + p2
                nc.vector.scalar_tensor_tensor(
                    mkap(cg[:], 20 * l0 + 16, [[20, nl], [1, 4]]),
                    mkap(prod[:], 8 * l0 + 4, [[8, nl], [1, 4]]),
                    0.5,
                    mkap(prod[:], 8 * l0, [[8, nl], [1, 4]]),
                    ALU.mult, ALU.add,
                )
                return prod

            def chain_part2(group, w, tag):
                """tanh(c) and h' = (T_o + 1) * tanh(c)."""
                l0, nl = group[0], len(group)
                tct = work.tile([128, 12], FP32, tag="tct" + tag,
                                name="tct" + tag, bufs=4)
                nc.scalar.activation(
                    mkap(tct[:], 4 * l0, [[4, nl], [1, 4]]),
                    mkap(cg[:], 20 * l0 + 16, [[20, nl], [1, 4]]),
                    AF.Tanh, scale=0.5,
                )
                # chunk-major h' writes (merged across layers): chunk 0 lands
                # first so next wave's kc0 matmuls can start immediately.
                for kc in range(2):
                    nc.vector.scalar_tensor_tensor(
                        mkap(hta[:], 992 * l0 + w * Bb + kc * TB,
                             [[992, nl], [1, Bb]]),
                        mkap(cg[:], 20 * l0 + 8 + kc * 2, [[20, nl], [1, 2]]),
                        1.0,
                        mkap(tct[:], 4 * l0 + kc * 2, [[4, nl], [1, 2]]),
                        ALU.add, ALU.mult,
                    )

            # xg calendar
            xg_cal = {}
            prologue_xg = []
            for l in range(3):
                for c in range(NSUB):
                    for j in range(8):
                        if l == 0:
                            w = SUB * (c - 1) + j
                        else:
                            w = LAG * l + SUB * c - 9 + j
                        if w < 0:
                            prologue_xg.append((l, c, j))
                        else:
                            xg_cal.setdefault(w, []).append((l, c, j))

            # FC calendar: token-tile mt ready at wave 64*mt+95
            fc_cal = {}
            for mt in range(3):
                for v in range(NVC):
                    fc_cal.setdefault(64 * mt + 96 + v, []).append((mt, v))
            # lse for mt0/mt1 inline, a few waves after the last chunk's exp
            lse_cal = {64 * mt + 96 + NVC + 4: mt for mt in range(2)}
            # subtract calendar for mt0/mt1
            sub_cal = {}
            for mt in range(2):
                for k in range(NRB):
                    sub_cal.setdefault(64 * mt + 96 + NVC + 7 + 2 * k, []).append((mt, k))

            for (l, c, j) in prologue_xg:
                emit_xg_piece(l, c, j)

            # fcW load: after the embedding/weight DMAs (first needed ~wave 96)
            for kc in range(2):
                for q in range(8):
                    nc.sync.dma_start(
                        fcw_sb[kc][:, q * 4000:(q + 1) * 4000],
                        fcWT_d[kc * 128:(kc + 1) * 128, q * 4000:(q + 1) * 4000])
            if has_fc_bias:
                nc.sync.dma_start(fcb_sb[:], fcb_d[:])

            for w in range(W_TOT):
                active = [l for l in range(3) if 0 <= w - LAG * l < T]
                g01 = [l for l in active if l < 2]
                g2 = [l for l in active if l == 2]
                P = (w // SUB) % 2
                s8 = w % SUB
                gbase = P * 1536 + s8 * 16

                emit_wh(g01, w, gbase)
                emit_wh(g2, w, gbase)
                if g01:
                    chain_part1(g01, w, gbase, "01")
                if g2:
                    chain_part1(g2, w, gbase, "2")
                fcs = fc_cal.get(w, ())
                for (mt, v) in fcs:
                    fc_mm(mt, v)
                    # the CAST fills the V-queue slot where the chain waits on
                    # tanh(c) from scalar, keeping it off the inter-wave path
                    fc_copy(mt, v, stage_state["ps"][(mt, v)])
                if g01:
                    chain_part2(g01, w, "01")
                if g2:
                    chain_part2(g2, w, "2")
                # DMA + 2000-col exp at the scalar-queue tail every 4th wave
                for (mt, v) in fcs:
                    fc_flush(mt, v)
                for (l, c, j) in xg_cal.get(w, ()):
                    emit_xg_piece(l, c, j)
                if w in lse_cal:
                    emit_lse(lse_cal[w])
                for (mt, k) in sub_cal.get(w, ()):
                    emit_sub(mt, k)

            wpool.release()
            gpp.release()

            # ---- Tail: mt2 subtract (overlaps mt3 FC), mt3 FC, lse3, subs
            emit_lse(2)
            for k in range(NRB):
                emit_sub(2, k)
            for v in range(NVC):
                fc_chunk(3, v)
            emit_lse(3)
            for k in range(NRB):
                emit_sub(3, k)

            rpool.release()
            stpool.release()
            fcpp.release()
            fcwpool.release()

    nc.compile()
    return nc


_nc_cache = {}


def _get_nc(has_lstm_bias, has_fc_bias):
    key = (has_lstm_bias, has_fc_bias)
    if key not in _nc_cache:
        _nc_cache[key] = build_nc(has_lstm_bias, has_fc_bias)
    return _nc_cache[key]


def prep_inputs(x, emb, Wi, Wh, bb, fcW, fcb):
    """Host-side shard + repack. Returns in_maps for the 8 cores.

    Gate rows reordered to [i,f,o,g].  Row scale 0.5 on i,f,o (sigmoid via
    tanh(x/2)); h-consuming weights additionally halved (h' = 2h); fcW halved.
    """
    perm = np.concatenate([np.arange(0, 512), np.arange(768, 1024),
                           np.arange(512, 768)])  # i,f | o | g
    rowscale = np.ones((G4, 1), np.float32)
    rowscale[0:768] = 0.5  # i,f,o rows: tanh(pre/2)
    shared = {
        "emb": np.ascontiguousarray(emb.astype(np.float32)),
        "fcWT": np.ascontiguousarray((fcW.T * 0.5).astype(np.float16)),
        "fcb": np.ascontiguousarray(fcb[None, :].astype(np.float16)),
    }
    for l in range(3):
        inscale = 1.0 if l == 0 else 0.5  # layers 1,2 consume h' = 2h
        shared[f"wiT{l}"] = np.ascontiguousarray(
            (Wi[l][perm] * rowscale * inscale).T.astype(np.float16))
        shared[f"whT{l}"] = np.ascontiguousarray(
            (Wh[l][perm] * rowscale * 0.5).T.astype(np.float16))
        shared[f"bvec{l}"] = np.ascontiguousarray(
            (bb[l][perm] * rowscale[:, 0])[None, :].astype(np.float16))
    in_maps = []
    for c in range(N_CORES):
        x_loc = x[c * B_LOC:(c + 1) * B_LOC, :]
        xids = np.ascontiguousarray(
            x_loc.T.reshape(-1, 1).astype(np.int32))  # [(t b), 1]
        m = dict(shared)
        m["xids"] = xids
        in_maps.append(m)
    return in_maps


def kernel(x, emb, Wi0, Wh0, b0, Wi1, Wh1, b1, Wi2, Wh2, b2, fcW, fcb,
           trace=False):
    x = np.asarray(x)
    bbs = [np.asarray(b0), np.asarray(b1), np.asarray(b2)]
    has_lstm_bias = bool(any(np.any(b) for b in bbs))
    has_fc_bias = bool(np.any(np.asarray(fcb)))
    nc = _get_nc(has_lstm_bias, has_fc_bias)
    in_maps = prep_inputs(
        np.asarray(x), np.asarray(emb),
        [np.asarray(Wi0), np.asarray(Wi1), np.asarray(Wi2)],
        [np.asarray(Wh0), np.asarray(Wh1), np.asarray(Wh2)],
        bbs, np.asarray(fcW), np.asarray(fcb))
    res = run_bass_kernel_spmd(nc, in_maps, core_ids=list(range(N_CORES)),
                               trace=trace)
    out = np.empty((B, T, V), np.float32)
    for c in range(N_CORES):
        oc = res.results[c]["out"].astype(np.float32).reshape(T, B_LOC, V)
        out[c * B_LOC:(c + 1) * B_LOC] = oc.transpose(1, 0, 2)
    kernel.last_results = res
    return out


# revision 38
# speedup vs baseline: 1.1012x; 1.0473x over previous
"""Trainium2 Bass kernel: 3-layer LSTM LM (embed -> 3xLSTM(H=256) -> FC 32000 -> log_softmax).

Strategy: data-parallel over batch across 8 cores (2 sequences per core).

v2 wave design:
- LSTM cell reformulated with tanh only: sigmoid(x) = (1+tanh(x/2))/2 with the
  1/2 folded into pre-scaled weights, and doubled states c' = 2c, h' = 2h
  (weights consuming h are pre-halved).  This keeps the whole recurrent chain
  in the `exp_and_others` ACT table set, so log-softmax exp-accumulation runs
  inline during the wavefront with no table switches.
- The per-wave nonlinear chain is split into two independent streams
  ({L0,L1} merged via strided APs, {L2}) so each stream's chain overlaps the
  other stream's matmuls of the next wave.
- Cell update uses fused scalar_tensor_tensor ops:
    T = tanh(gates)            (ACT, all 16 cols/layer; weights pre-scaled)
    [p2,p1] = (T[i,f] + 1) * [T_g, c']      (one strided STT)
    c'_new  = 0.5*p1 + p2                    (STT)
    tc      = tanh(0.5 * c'_new)             (ACT, free input scale)
    h'      = (T_o + 1) * tc                 (STT)
- FC logits: matmul -> PSUM; exp-accumulated directly from PSUM (scalar) and
  staged to DRAM fp16 via gpsimd cast-DMA (no stage copies).  lse for token
  tiles 0/1 computed inline (Ln costs a table-switch pair each); their
  subtract+rewrite also runs inline.  Tiles 2/3 finish in the tail.
"""

import sys

sys.path.insert(0, "/opt/trn_rl_repo")

import numpy as np

import concourse.bass as bass
import concourse.mybir as mybir
import concourse.tile as tile
from concourse import bacc
from concourse.bass_utils import run_bass_kernel_spmd
from concourse.masks import make_identity
from concourse.tile import add_dep_helper

# Problem dims
V = 32000
E = 200
H = 256
B = 16
T = 256
N_CORES = 8
B_LOC = B // N_CORES  # 2 sequences per core
Bb = B_LOC
G4 = 4 * H  # 1024 gate width
TB = T * Bb  # 512 token-cols per h chunk

LAG = 16     # inter-layer lag in steps
SUB = 8      # xg-precompute granularity (steps per psum sub-chunk)
NSUB = T // SUB
W_TOT = T + 2 * LAG  # 288 waves

VC = 500           # FC vocab chunk (one PSUM bank)
NVC = V // VC      # 64
RBW = 2000         # readback/subtract chunk
NRB = V // RBW     # 16

FP16 = mybir.dt.float16
FP32 = mybir.dt.float32
AF = mybir.ActivationFunctionType
ALU = mybir.AluOpType
LAYER_DIMS = [E, H, H]


def mkap(tile_ap, off, dims):
    """Custom strided AP on a tile: off in elements, dims=[[step,count],...]."""
    return bass.AP(tile_ap.tensor, off,
                   [list(tile_ap.ap[0])] + [list(d) for d in dims])


def ksizes(dim):
    out = []
    while dim > 0:
        out.append(min(dim, 128))
        dim -= 128
    return out


def build_nc(has_lstm_bias=False, has_fc_bias=False):
    ntok = T * Bb  # 512 tokens per core
    n_mt = ntok // 128  # 4 fc token tiles

    nc = bacc.Bacc("TRN2", target_bir_lowering=False, debug=False,
                   num_devices=N_CORES)

    xids_d = nc.dram_tensor("xids", [ntok, 1], mybir.dt.int32, kind="ExternalInput")
    emb_d = nc.dram_tensor("emb", [V, E], FP32, kind="ExternalInput")
    wiT_d = [nc.dram_tensor(f"wiT{l}", [LAYER_DIMS[l], G4], FP16, kind="ExternalInput")
             for l in range(3)]
    whT_d = [nc.dram_tensor(f"whT{l}", [H, G4], FP16, kind="ExternalInput")
             for l in range(3)]
    bvec_d = [nc.dram_tensor(f"bvec{l}", [1, G4], FP16, kind="ExternalInput")
              for l in range(3)]
    fcWT_d = nc.dram_tensor("fcWT", [H, V], FP16, kind="ExternalInput")
    fcb_d = nc.dram_tensor("fcb", [1, V], FP16, kind="ExternalInput")
    out_d = nc.dram_tensor("out", [ntok, V], FP16, kind="ExternalOutput")

    with tile.TileContext(nc, num_cores=N_CORES) as tc:
        with (
            tc.tile_pool(name="state", bufs=1) as spool,
            tc.tile_pool(name="work", bufs=4) as work,
        ):
            # ---- persistent state ----
            hta = spool.tile([128, 6 * TB], FP16, tag="hta", name="hta")

            def ht_off(l, kc):
                return l * 2 * TB + kc * TB
            # cg: per layer 20 cols: [T16 (i,f,o,g tanh outputs) | c' (4)]
            cg = spool.tile([128, 60], FP32, tag="cg", name="cg")
            nc.vector.memset(cg[:], 0.0)
            zacc = spool.tile([128, 4 * 2 * NRB], FP32, tag="zacc", name="zacc")
            neglse = spool.tile([128, 4], FP32, tag="neglse", name="neglse")
            etrash = spool.tile([128, 4 * VC], FP16, tag="etrash", name="etrash")

            fcwpool = tc.alloc_tile_pool(name="fcw", bufs=1)
            fcw_sb = [fcwpool.tile([128, V], FP16, tag=f"fcw{kc}", name=f"fcw{kc}")
                      for kc in range(2)]
            fcb_sb = None
            if has_fc_bias:
                fcb_sb = fcwpool.tile([1, V], FP16, tag="fcb", name="fcb")

            stpool = tc.alloc_tile_pool(name="stage", bufs=4)
            rpool = tc.alloc_tile_pool(name="rb", bufs=4)
            wpool = tc.alloc_tile_pool(name="weights", bufs=1)
            # ---- Phase 0: LSTM weights to SBUF ----
            wiT_sb = []
            whT_sb = []
            bvec_sb = []
            for l in range(3):
                ks = ksizes(LAYER_DIMS[l])
                wi = wpool.tile([128, len(ks) * G4], FP16, tag=f"wiT{l}",
                                name=f"wiT{l}")
                for kc, ksz in enumerate(ks):
                    nc.sync.dma_start(
                        wi[0:ksz, kc * G4:(kc + 1) * G4],
                        wiT_d[l][kc * 128:kc * 128 + ksz, :],
                    )
                wiT_sb.append(wi)
                wh = wpool.tile([128, 2 * G4], FP16, tag=f"whT{l}", name=f"whT{l}")
                for kc in range(2):
                    nc.sync.dma_start(
                        wh[:, kc * G4:(kc + 1) * G4],
                        whT_d[l][kc * 128:(kc + 1) * 128, :],
                    )
                whT_sb.append(wh)
                if has_lstm_bias:
                    bv = wpool.tile([1, G4], FP16, tag=f"bvec{l}", name=f"bvec{l}")
                    nc.sync.dma_start(bv[:], bvec_d[l][:])
                    bvec_sb.append(bv)
                else:
                    bvec_sb.append(None)

            ones_sb = wpool.tile([1, 16], FP16, tag="ones", name="ones")
            nc.vector.memset(ones_sb[:], 1.0)
            onesf = spool.tile([1, 128], FP32, tag="onesf", name="onesf")
            nc.vector.memset(onesf[:], 1.0)
            ident = wpool.tile([128, 128], FP32, tag="ident", name="ident")
            make_identity(nc, ident[:])
            zrhs = wpool.tile([128, Bb], FP16, tag="zrhs", name="zrhs")
            nc.vector.memset(zrhs[:], 0.0)
            xT = wpool.tile([128, 2 * TB], FP16, tag="xT", name="xT")

            # ---- Phase 1: embedding gather + transpose into xT ----
            eks = ksizes(E)
            with tc.tile_pool(name="embps", bufs=2, space="PSUM") as eps:
                for gt in range(ntok // 128):
                    idt = work.tile([128, 1], mybir.dt.int32, tag="ids", name="ids")
                    nc.sync.dma_start(idt[:], xids_d[gt * 128:(gt + 1) * 128, :])
                    gat = work.tile([128, E], FP32, tag="gather", name="gather", bufs=2)
                    nc.gpsimd.indirect_dma_start(
                        out=gat[:],
                        out_offset=None,
                        in_=emb_d[:, :],
                        in_offset=bass.IndirectOffsetOnAxis(ap=idt[:, :1], axis=0),
                    )
                    for kc, ksz in enumerate(eks):
                        tp = eps.tile([128, 128], FP32, tag="tpsum", name="tpsum")
                        nc.tensor.transpose(
                            tp[0:ksz, 0:128],
                            gat[:, kc * 128:kc * 128 + ksz],
                            ident[:],
                        )
                        nc.vector.tensor_copy(
                            xT[0:ksz, kc * TB + gt * 128:kc * TB + (gt + 1) * 128],
                            tp[0:ksz, 0:128],
                        )

            # ---- Phase 2: wavefront with 2 chain streams + inline FC ----
            fcpp = tc.alloc_tile_pool(name="fcps", bufs=2, space="PSUM")
            gpp = tc.alloc_tile_pool(name="gps", bufs=1, space="PSUM")
            gp = gpp.tile([128, 3072], FP32, tag="gp", name="gp")
            openers = {}

            def emit_xg_piece(l, c, j):
                """Emit 2 of the 16 xg matmuls for (layer l, sub-chunk c)."""
                ks = ksizes(LAYER_DIMS[l])
                sb = (c % 2) * 1536 + l * 512
                mms = []
                for t in (2 * j, 2 * j + 1):
                    m, kc = t % 8, t // 8
                    ksz = ks[kc]
                    if l == 0:
                        rhs = xT[0:ksz, kc * TB + c * 16:kc * TB + (c + 1) * 16]
                    else:
                        rhs = hta[0:ksz,
                                  ht_off(l - 1, kc) + c * 16:ht_off(l - 1, kc) + (c + 1) * 16]
                    out = mkap(gp[:], sb + m * Bb, [[16, SUB], [1, Bb]])
                    is_open = (m == 0 and kc == 0)
                    mm = nc.tensor.matmul(
                        out,
                        lhsT=wiT_sb[l][0:ksz,
                                       kc * G4 + m * 128:kc * G4 + (m + 1) * 128],
                        rhs=rhs,
                        start=is_open,
                        stop=False,
                        skip_group_check=True,
                    )
                    if is_open:
                        openers[(l, c)] = mm.ins
                    else:
                        add_dep_helper(mm.ins, openers[(l, c)], sync=False,
                                       reason="slot opener order")
                    mms.append(mm)
                if has_lstm_bias and j == 7:
                    for m in range(8):
                        mm = nc.tensor.matmul(
                            mkap(gp[:], sb + m * Bb, [[16, SUB], [1, Bb]]),
                            lhsT=bvec_sb[l][:, m * 128:(m + 1) * 128],
                            rhs=ones_sb[:, 0:16],
                            start=False,
                            stop=False,
                            skip_group_check=True,
                        )
                        add_dep_helper(mm.ins, openers[(l, c)], sync=False,
                                       reason="slot opener order")
                        mms.append(mm)
                return mms

            stage_state = {}

            def fc_mm(mt, v):
                """FC matmul for token-tile mt, vocab cols [v*VC, +VC) -> PSUM."""
                vs = v * VC
                ps = fcpp.tile([128, VC], FP32, tag="fcpsum", name="fcpsum")
                stage_state.setdefault("ps", {})[(mt, v)] = ps
                first = None
                for kc in range(2):
                    mm = nc.tensor.matmul(
                        ps[:],
                        lhsT=hta[:, ht_off(2, kc) + mt * 128:ht_off(2, kc) + (mt + 1) * 128],
                        rhs=fcw_sb[kc][:, vs:vs + VC],
                        start=(kc == 0),
                        stop=(kc == 1 and not has_fc_bias),
                        skip_group_check=True,
                    )
                    if first is None:
                        first = mm
                    last = mm
                if has_fc_bias:
                    last = nc.tensor.matmul(
                        ps[:],
                        lhsT=onesf[:, 0:128],
                        rhs=fcb_sb[:, vs:vs + VC],
                        start=False,
                        stop=True,
                        skip_group_check=True,
                    )
                return ps, first, last

            def fc_copy(mt, v, ps):
                """fp32 PSUM -> fp16 stage slice on Vector."""
                if v % 4 == 0:
                    stage_state["tile"] = stpool.tile([128, 4 * VC], FP16,
                                                      tag="fcstage", name="fcstage")
                st = stage_state["tile"]
                return nc.vector.tensor_copy(
                    st[:, (v % 4) * VC:(v % 4 + 1) * VC], ps[:])

            exp_pending = {}  # wave -> [(mt, k, half, stage_tile)]

            def fc_flush(mt, v, w):
                """Every 4th chunk: DMA the 2000-col stage out; schedule its
                exp-accum as two 1000-col halves on the next two waves."""
                if v % 4 != 3:
                    return
                st = stage_state["tile"]
                k = v // 4
                dout = nc.sync.dma_start(
                    out_d[mt * 128:(mt + 1) * 128, k * 4 * VC:(k + 1) * 4 * VC],
                    st[:],
                )
                stage_state.setdefault("out", {})[(mt, k)] = dout.ins
                for half in range(2):
                    exp_pending.setdefault(w + 1 + half, []).append((mt, k, half, st))

            def emit_exp_half(mt, k, half, st):
                return nc.scalar.activation(
                    etrash[:, 0:2 * VC], st[:, half * 2 * VC:(half + 1) * 2 * VC],
                    AF.Exp,
                    accum_out=zacc[:, mt * 2 * NRB + 2 * k + half:
                                   mt * 2 * NRB + 2 * k + half + 1],
                )

            def fc_chunk(mt, v, w):
                ps, _, _ = fc_mm(mt, v)
                fc_copy(mt, v, ps)
                fc_flush(mt, v, w)

            def emit_lse(mt):
                """zacc[mt] -> neglse[:, mt].  Ln costs a table-switch pair."""
                zs = work.tile([128, 1], FP32, tag="zsum", name="zsum")
                nc.vector.tensor_reduce(
                    zs[:], zacc[:, mt * 2 * NRB:(mt + 1) * 2 * NRB],
                    op=ALU.add, axis=mybir.AxisListType.X,
                )
                lse = work.tile([128, 1], FP32, tag="lse", name="lse")
                nc.scalar.activation(lse[:], zs[:], AF.Ln)
                nc.vector.tensor_scalar_mul(neglse[:, mt:mt + 1], lse[:], -1.0)

            def emit_sub(mt, k):
                """Read back out_d chunk, add -lse, rewrite."""
                rb = rpool.tile([128, RBW], FP16, tag="rb", name="rb")
                din = nc.sync.dma_start(
                    rb[:], out_d[mt * 128:(mt + 1) * 128, k * RBW:(k + 1) * RBW])
                src = stage_state.get("out", {}).get((mt, k))
                if src is not None:
                    add_dep_helper(din.ins, src, sync=True,
                                   reason="out_d staged-write before readback")
                nc.vector.tensor_scalar_add(rb[:], rb[:], neglse[:, mt:mt + 1])
                nc.sync.dma_start(
                    out_d[mt * 128:(mt + 1) * 128, k * RBW:(k + 1) * RBW], rb[:])

            def emit_wh(group, w, gbase):
                # kc-major so the kc0 matmuls depend only on the h' chunk-0
                # write of the previous wave (which lands first).
                first = last = None
                for kc in range(2):
                    for l in group:
                        tl = w - LAG * l
                        sb = gbase + l * 512
                        if tl == 0:
                            rhs = zrhs[:, 0:Bb]
                        else:
                            rhs = hta[:, ht_off(l, kc) + (tl - 1) * Bb:
                                      ht_off(l, kc) + tl * Bb]
                        for m in range(8):
                            mm = nc.tensor.matmul(
                                gp[:, sb + m * Bb:sb + (m + 1) * Bb],
                                lhsT=whT_sb[l][:, kc * G4 + m * 128:
                                               kc * G4 + (m + 1) * 128],
                                rhs=rhs,
                                start=False,
                                stop=(kc == 1),
                                skip_group_check=True,
                            )
                            if first is None:
                                first = mm
                            last = mm
                return first, last

            def chain_part1(group, w, gbase, tag):
                """tanh of gates, then fused cell update -> new c'.
                Returns (T_act, p_stt, c_stt) instructions."""
                l0, nl = group[0], len(group)
                t_act = nc.scalar.activation(
                    mkap(cg[:], 20 * l0, [[20, nl], [1, 16]]),
                    mkap(gp[:], gbase + l0 * 512, [[512, nl], [1, 16]]),
                    AF.Tanh,
                )
                prod = work.tile([128, 24], FP32, tag="prod" + tag,
                                 name="prod" + tag, bufs=4)
                # [p2,p1] = (T[i,f] + 1) * [T_g, c']
                p_stt = nc.vector.scalar_tensor_tensor(
                    mkap(prod[:], 8 * l0, [[8, nl], [1, 8]]),
                    mkap(cg[:], 20 * l0, [[20, nl], [1, 8]]),
                    1.0,
                    mkap(cg[:], 20 * l0 + 12, [[20, nl], [1, 8]]),
                    ALU.add, ALU.mult,
                )
                # c'_new = 0.5*p1 + p2
                c_stt = nc.vector.scalar_tensor_tensor(
                    mkap(cg[:], 20 * l0 + 16, [[20, nl], [1, 4]]),
                    mkap(prod[:], 8 * l0 + 4, [[8, nl], [1, 4]]),
                    0.5,
                    mkap(prod[:], 8 * l0, [[8, nl], [1, 4]]),
                    ALU.mult, ALU.add,
                )
                return t_act, p_stt, c_stt

            def chain_part2(group, w, tag):
                """tanh(c) and h' = (T_o + 1) * tanh(c).
                Returns (tc_act, h_stt_chunk0, h_stt_chunk1)."""
                l0, nl = group[0], len(group)
                tct = work.tile([128, 12], FP32, tag="tct" + tag,
                                name="tct" + tag, bufs=4)
                tc_act = nc.scalar.activation(
                    mkap(tct[:], 4 * l0, [[4, nl], [1, 4]]),
                    mkap(cg[:], 20 * l0 + 16, [[20, nl], [1, 4]]),
                    AF.Tanh, scale=0.5,
                )
                # chunk-major h' writes (merged across layers): chunk 0 lands
                # first so next wave's kc0 matmuls can start immediately.
                h_stts = []
                for kc in range(2):
                    h_stts.append(nc.vector.scalar_tensor_tensor(
                        mkap(hta[:], 992 * l0 + w * Bb + kc * TB,
                             [[992, nl], [1, Bb]]),
                        mkap(cg[:], 20 * l0 + 8 + kc * 2, [[20, nl], [1, 2]]),
                        1.0,
                        mkap(tct[:], 4 * l0 + kc * 2, [[4, nl], [1, 2]]),
                        ALU.add, ALU.mult,
                    ))
                return tc_act, h_stts[0], h_stts[1]

            # xg calendar
            xg_cal = {}
            prologue_xg = []
            for l in range(3):
                for c in range(NSUB):
                    for j in range(8):
                        if l == 0:
                            w = SUB * (c - 1) + j
                        else:
                            w = LAG * l + SUB * c - 9 + j
                        if w < 0:
                            prologue_xg.append((l, c, j))
                        else:
                            xg_cal.setdefault(w, []).append((l, c, j))

            # FC calendar: token-tile mt ready at wave 64*mt+95
            fc_cal = {}
            for mt in range(3):
                for v in range(NVC):
                    fc_cal.setdefault(64 * mt + 96 + v, []).append((mt, v))
            # lse for mt0/mt1 inline, a few waves after the last chunk's exp
            lse_cal = {64 * mt + 96 + NVC + 4: mt for mt in range(2)}
            # subtract calendar for mt0/mt1
            sub_cal = {}
            for mt in range(2):
                for k in range(NRB):
                    sub_cal.setdefault(64 * mt + 96 + NVC + 7 + 2 * k, []).append((mt, k))

            for (l, c, j) in prologue_xg:
                emit_xg_piece(l, c, j)

            # fcW load: after the embedding/weight DMAs (first needed ~wave 96)
            for kc in range(2):
                for q in range(8):
                    nc.sync.dma_start(
                        fcw_sb[kc][:, q * 4000:(q + 1) * 4000],
                        fcWT_d[kc * 128:(kc + 1) * 128, q * 4000:(q + 1) * 4000])
            if has_fc_bias:
                nc.sync.dma_start(fcb_sb[:], fcb_d[:])

            def order(b, a):
                """b executes after a in its engine queue (order-only edge)."""
                if a is not None and b is not None:
                    add_dep_helper(b.ins, a.ins, sync=False, reason="wave order")

            prev_filler_last = None  # last tensor filler of previous wave
            for w in range(W_TOT):
                active = [l for l in range(3) if 0 <= w - LAG * l < T]
                g01 = [l for l in active if l < 2]
                g2 = [l for l in active if l == 2]
                P = (w // SUB) % 2
                s8 = w % SUB
                gbase = P * 1536 + s8 * 16

                wh01_f = wh01_l = wh2_f = wh2_l = None
                if g01:
                    wh01_f, wh01_l = emit_wh(g01, w, gbase)
                if g2:
                    wh2_f, wh2_l = emit_wh(g2, w, gbase)
                # tensor order: [Wh g01][Wh g2][fc mm][xg fillers], and the
                # new wave's Wh block after the previous wave's fillers.
                order(wh01_f or wh2_f, prev_filler_last)
                order(wh2_f, wh01_l)

                t01 = p01 = c01 = t2 = p2 = c2 = None
                if g01:
                    t01, p01, c01 = chain_part1(g01, w, gbase, "01")
                if g2:
                    t2, p2, c2 = chain_part1(g2, w, gbase, "2")
                order(t2, t01)
                order(p2, c01)

                fcs = fc_cal.get(w, ())
                cast = None
                fcm_l = None
                for (mt, v) in fcs:
                    ps, fcm_f, fcm_l = fc_mm(mt, v)
                    order(fcm_f, wh2_l or wh01_l)
                    cast = fc_copy(mt, v, ps)

                tc01 = h01a = h01b = tc2 = h2a = h2b = None
                if g01:
                    tc01, h01a, h01b = chain_part2(g01, w, "01")
                if g2:
                    tc2, h2a, h2b = chain_part2(g2, w, "2")
                # scalar order: T01 -> T2 -> tc01 -> tc2
                order(tc01, t2 or t01)
                order(tc2, tc01 or t2)
                # V order: ..c2 -> h01a -> h01b -> h2a -> h2b -> cast
                order(h01a, c2 or c01)
                order(h2a, h01b or c2)
                order(cast, h2b or h01b or c2 or c01)

                for (mt, v) in fcs:
                    fc_flush(mt, v, w)
                # exp halves from flushes 1-2 waves ago, after this wave's ACTs
                for (mt, k, half, st) in exp_pending.pop(w, ()):
                    e = emit_exp_half(mt, k, half, st)
                    order(e, tc2 or tc01 or t2 or t01)

                xg_l = None
                for (l, c, j) in xg_cal.get(w, ()):
                    mms = emit_xg_piece(l, c, j)
                    if xg_l is None and mms:
                        order(mms[0], fcm_l or wh2_l or wh01_l)
                    if mms:
                        xg_l = mms[-1]
                prev_filler_last = xg_l or fcm_l or wh2_l or wh01_l

                if w in lse_cal:
                    emit_lse(lse_cal[w])
                for (mt, k) in sub_cal.get(w, ()):
                    emit_sub(mt, k)

            wpool.release()
            gpp.release()

            # ---- Tail: mt2 subtract (overlaps mt3 FC), mt3 FC, lse3, subs
            def drain_exps():
                for wk in sorted(exp_pending):
                    for (mt, k, half, st) in exp_pending.pop(wk):
                        emit_exp_half(mt, k, half, st)

            drain_exps()  # mt2's last flushes
            emit_lse(2)
            for k in range(NRB):
                emit_sub(2, k)
            for v in range(NVC):
                fc_chunk(3, v, W_TOT + v)
            drain_exps()
            emit_lse(3)
            for k in range(NRB):
                emit_sub(3, k)

            rpool.release()
            stpool.release()
            fcpp.release()
            fcwpool.release()

    nc.compile()
    return nc


_nc_cache = {}


def _get_nc(has_lstm_bias, has_fc_bias):
    key = (has_lstm_bias, has_fc_bias)
    if key not in _nc_cache:
        _nc_cache[key] = build_nc(has_lstm_bias, has_fc_bias)
    return _nc_cache[key]


def prep_inputs(x, emb, Wi, Wh, bb, fcW, fcb):
    """Host-side shard + repack. Returns in_maps for the 8 cores.

    Gate rows reordered to [i,f,o,g].  Row scale 0.5 on i,f,o (sigmoid via
    tanh(x/2)); h-consuming weights additionally halved (h' = 2h); fcW halved.
    """
    perm = np.concatenate([np.arange(0, 512), np.arange(768, 1024),
                           np.arange(512, 768)])  # i,f | o | g
    rowscale = np.ones((G4, 1), np.float32)
    rowscale[0:768] = 0.5  # i,f,o rows: tanh(pre/2)
    shared = {
        "emb": np.ascontiguousarray(emb.astype(np.float32)),
        "fcWT": np.ascontiguousarray((fcW.T * 0.5).astype(np.float16)),
        "fcb": np.ascontiguousarray(fcb[None, :].astype(np.float16)),
    }
    for l in range(3):
        inscale = 1.0 if l == 0 else 0.5  # layers 1,2 consume h' = 2h
        shared[f"wiT{l}"] = np.ascontiguousarray(
            (Wi[l][perm] * rowscale * inscale).T.astype(np.float16))
        shared[f"whT{l}"] = np.ascontiguousarray(
            (Wh[l][perm] * rowscale * 0.5).T.astype(np.float16))
        shared[f"bvec{l}"] = np.ascontiguousarray(
            (bb[l][perm] * rowscale[:, 0])[None, :].astype(np.float16))
    in_maps = []
    for c in range(N_CORES):
        x_loc = x[c * B_LOC:(c + 1) * B_LOC, :]
        xids = np.ascontiguousarray(
            x_loc.T.reshape(-1, 1).astype(np.int32))  # [(t b), 1]
        m = dict(shared)
        m["xids"] = xids
        in_maps.append(m)
    return in_maps


def kernel(x, emb, Wi0, Wh0, b0, Wi1, Wh1, b1, Wi2, Wh2, b2, fcW, fcb,
           trace=False):
    x = np.asarray(x)
    bbs = [np.asarray(b0), np.asarray(b1), np.asarray(b2)]
    has_lstm_bias = bool(any(np.any(b) for b in bbs))
    has_fc_bias = bool(np.any(np.asarray(fcb)))
    nc = _get_nc(has_lstm_bias, has_fc_bias)
    in_maps = prep_inputs(
        np.asarray(x), np.asarray(emb),
        [np.asarray(Wi0), np.asarray(Wi1), np.asarray(Wi2)],
        [np.asarray(Wh0), np.asarray(Wh1), np.asarray(Wh2)],
        bbs, np.asarray(fcW), np.asarray(fcb))
    res = run_bass_kernel_spmd(nc, in_maps, core_ids=list(range(N_CORES)),
                               trace=trace)
    out = np.empty((B, T, V), np.float32)
    for c in range(N_CORES):
        oc = res.results[c]["out"].astype(np.float32).reshape(T, B_LOC, V)
        out[c * B_LOC:(c + 1) * B_LOC] = oc.transpose(1, 0, 2)
    kernel.last_results = res
    return out
